# revision 5
# baseline (speedup 1.0000x reference)
"""DigitCaps dynamic-routing kernel for 8 Trainium2 NeuronCores.

Decomposition (validated against the jax reference in a numpy mock):
  u_hat[b,r,c,o] = sum_i W[r,c,o,i] x[b,r,i]   (never materialized)
  s[b,c,o]   = sum_{r,i} x[b,r,i] * cexp[r,c] * W[r,c,o,i] / sigma[c]
  a[r,c]     = (1/B) sum_{i,o} W[r,c,o,i] * G[r,i,c,o],
               G[(r,i),(c,o)] = sum_b x[b,r,i] v[b,c,o]
  softmax over r is computed unnormalized (cexp = exp(b), safe: |b| < 1);
  the divide by sigma[c] = sum_r exp(b[r,c]) folds into the squash.

Sharding: 8-way over routes R (288 each). Each core holds its W shard and
the full batch of x for its routes, so `a` needs no collective. The only
cross-core exchange is the per-iteration AllReduce of the partial
s[b,(c,o)] (plus the 10 local sigma values in the same payload). The final
iteration's s-partials are returned per core; the host sums them and
applies the (tiny) squash while unsharding.

SBUF row space per core: 18 tiles of 128 rows; tile t row (i*16+j) holds
(r = 288*core + 16*t + j, i) — i-major so the exp(b) -> row expansion DMA
writes contiguous partition blocks.
"""
import os
import sys

import numpy as np

_REPO = "/opt/trn_rl_repo"
if _REPO not in sys.path:
    sys.path.insert(0, _REPO)

import concourse.bass as bass  # noqa: E402,F401
import concourse.tile as tile  # noqa: E402
from concourse import bacc, mybir  # noqa: E402
from concourse import bass_utils  # noqa: E402

B, R, C, O, I = 256, 2304, 10, 16, 8
NCORE = 8
RL = R // NCORE          # 288 routes per core
KI = RL * I              # 2304 (r,i) rows per core
NT = KI // 128           # 18 tiles of 128 (r,i) rows
CO = C * O               # 160
NITER = 3
F32 = mybir.dt.float32
SIG_OFF = B * CO         # float offset of the sigma slot in the AR payload
PAY = SIG_OFF + 16       # AR payload floats (s parts + sigma + pad)

_CACHE = {"nc": None}
LAST_RESULT = None


def _build():
    nc = bacc.Bacc("TRN2", target_bir_lowering=False, debug=False, num_devices=NCORE)

    xT_d = nc.dram_tensor("xT", [KI, B], F32, kind="ExternalInput")
    xb_d = nc.dram_tensor("xb", [B, KI], F32, kind="ExternalInput")
    W_d = nc.dram_tensor("Wl", [KI, CO], F32, kind="ExternalInput")
    oblk_d = nc.dram_tensor("ones_blk", [128, 16], F32, kind="ExternalInput")
    o96_d = nc.dram_tensor("ones96", [96, 1], F32, kind="ExternalInput")
    o1_d = nc.dram_tensor("ones1", [1, 128], F32, kind="ExternalInput")
    sout_d = nc.dram_tensor("sout", [B, CO], F32, kind="ExternalOutput")
    sigout_d = nc.dram_tensor("sigout", [1, 16], F32, kind="ExternalOutput")

    with tile.TileContext(nc) as tc:
        with tc.tile_pool(name="big", bufs=1) as big, \
             tc.tile_pool(name="sml", bufs=2) as sml, \
             tc.tile_pool(name="ps_s", bufs=2, space="PSUM") as ps_s, \
             tc.tile_pool(name="ps_g", bufs=3, space="PSUM") as ps_g, \
             tc.tile_pool(name="ps_m", bufs=1, space="PSUM") as ps_m, \
             tc.tile_pool(name="dram", bufs=2, space="DRAM") as dram:

            # ---- preload ----
            W_sb = big.tile([128, NT, CO], F32, tag="W")
            nc.sync.dma_start(out=W_sb[:], in_=W_d.ap().rearrange("(t p) f -> p t f", p=128))
            xT_sb = big.tile([128, NT, B], F32, tag="xT")
            nc.sync.dma_start(out=xT_sb[:], in_=xT_d.ap().rearrange("(t p) b -> p t b", p=128))
            xb_sb = big.tile([128, 2, KI], F32, tag="xb")
            nc.sync.dma_start(out=xb_sb[:], in_=xb_d.ap().rearrange("(h p) k -> p h k", p=128))
            oblk_sb = big.tile([128, 16], F32, tag="oblk")
            nc.sync.dma_start(out=oblk_sb[:], in_=oblk_d.ap())
            o96_sb = big.tile([96, 1], F32, tag="o96")
            nc.sync.dma_start(out=o96_sb[:], in_=o96_d.ap())
            o1_sb = big.tile([1, 128], F32, tag="o1")
            nc.sync.dma_start(out=o1_sb[:], in_=o1_d.ap())

            b_sb = big.tile([96, 3, 10], F32, tag="b_state")
            nc.vector.memset(b_sb[:], 0.0)

            M_sb = big.tile([128, NT, CO], F32, tag="M")
            Gbig = big.tile([128, NT, CO], F32, tag="G")
            Pp = big.tile([128, NT, CO], F32, tag="Pp")
            P2 = big.tile([128, NT, C], F32, tag="P2")
            cE = big.tile([128, NT, C], F32, tag="cE")

            for it in range(NITER):
                # ---- unnormalized softmax from b state (iters >= 1) ----
                if it > 0:
                    exp_sb = sml.tile([96, 3, 10], F32, tag="exp")
                    nc.scalar.activation(out=exp_sb[:], in_=b_sb[:],
                                         func=mybir.ActivationFunctionType.Exp)
                    sig_ps = ps_m.tile([1, 30], F32, tag="sigps")
                    nc.tensor.matmul(sig_ps[:], lhsT=o96_sb[:],
                                     rhs=exp_sb[:].rearrange("p t c -> p (t c)"),
                                     start=True, stop=True)
                    sig_sb = sml.tile([1, 10], F32, tag="sigloc")
                    nc.vector.tensor_reduce(
                        out=sig_sb[:],
                        in_=sig_ps[:].rearrange("p (t c) -> p c t", c=10),
                        axis=mybir.AxisListType.X, op=mybir.AluOpType.add)
                    # route exp values to the expanded [(r,i), c] row layout
                    # (i-major within each tile) via a DRAM bounce
                    c_dram = dram.tile([RL, 10], F32, tag="c_dram")
                    nc.sync.dma_start(
                        out=c_dram[:].rearrange("(t p) c -> p t c", p=96),
                        in_=exp_sb[:])
                    for i_ in range(I):
                        nc.sync.dma_start(
                            out=cE[16 * i_:16 * i_ + 16, :, :],
                            in_=c_dram[:].rearrange("(t j) c -> j t c", j=16))
                    # M = W * cE (broadcast over o), chunked for PE overlap
                    for ch in range(3):
                        tsl = slice(6 * ch, 6 * ch + 6)
                        nc.vector.tensor_tensor(
                            out=M_sb[:, tsl, :].rearrange("p t (c o) -> p t c o", o=O),
                            in0=W_sb[:, tsl, :].rearrange("p t (c o) -> p t c o", o=O),
                            in1=cE[:, tsl, :, None].to_broadcast((128, 6, C, O)),
                            op=mybir.AluOpType.mult)
                else:
                    sig_sb = sml.tile([1, 10], F32, tag="sigloc")
                    nc.vector.memset(sig_sb[:], float(RL))

                # ---- s partial matmuls ----
                rhs_src = W_sb if it == 0 else M_sb
                psum_s = [ps_s.tile([128, CO], F32, tag="psum_s", name=f"psum_s{_h}") for _h in range(2)]
                for h in range(2):
                    for t in range(NT):
                        nc.tensor.matmul(
                            psum_s[h][:],
                            lhsT=xT_sb[:, t, 128 * h:128 * h + 128],
                            rhs=rhs_src[:, t, :],
                            start=(t == 0), stop=(t == NT - 1))

                sp_sb = sml.tile([128, 2, CO], F32, tag="sp_sb")
                for h in range(2):
                    nc.scalar.copy(out=sp_sb[:, h, :], in_=psum_s[h][:])

                if it == NITER - 1:
                    nc.sync.dma_start(
                        out=sout_d.ap().rearrange("(h p) f -> p h f", p=128),
                        in_=sp_sb[:])
                    nc.sync.dma_start(out=sigout_d.ap()[:, 0:10], in_=sig_sb[:])
                    break

                # ---- AllReduce payload: s parts + sigma ----
                ar_in = dram.tile([PAY], F32, tag="ar_in")
                ar_out = dram.tile([PAY], F32, tag="ar_out", addr_space="Shared")
                nc.sync.dma_start(
                    out=ar_in[0:B * CO].rearrange("(h p f) -> p h f", p=128, f=CO),
                    in_=sp_sb[:])
                nc.sync.dma_start(
                    out=ar_in[SIG_OFF:SIG_OFF + 10].rearrange("(p f) -> p f", p=1),
                    in_=sig_sb[:])
                nc.gpsimd.collective_compute(
                    "AllReduce", mybir.AluOpType.add,
                    replica_groups=[list(range(NCORE))],
                    ins=[ar_in.opt()], outs=[ar_out.opt()])

                s_sb = sml.tile([128, 2, CO], F32, tag="s_sb")
                nc.sync.dma_start(
                    out=s_sb[:],
                    in_=ar_out[0:B * CO].rearrange("(h p f) -> p h f", p=128, f=CO))
                sig_all = sml.tile([1, 10], F32, tag="sig_all")
                nc.sync.dma_start(
                    out=sig_all[:],
                    in_=ar_out[SIG_OFF:SIG_OFF + 10].rearrange("(p f) -> p f", p=1))

                # ---- squash: v = s_u * sqrt(sq)/(1+sq) / sigma,
                #      sq = (sum_o s_u^2)/sigma^2 ----
                sigB_ps = ps_m.tile([128, 10], F32, tag="sigB")
                nc.tensor.matmul(sigB_ps[:], lhsT=o1_sb[:], rhs=sig_all[:],
                                 start=True, stop=True)
                r1 = sml.tile([128, 10], F32, tag="r1")
                nc.vector.reciprocal(out=r1[:], in_=sigB_ps[:])
                ssq = sml.tile([128, 2, CO], F32, tag="ssq")
                nc.vector.tensor_tensor(out=ssq[:], in0=s_sb[:], in1=s_sb[:],
                                        op=mybir.AluOpType.mult)
                t2 = sml.tile([128, 2, C], F32, tag="t2")
                nc.vector.tensor_reduce(
                    out=t2[:], in_=ssq[:].rearrange("p h (c o) -> p h c o", o=O),
                    axis=mybir.AxisListType.X, op=mybir.AluOpType.add)
                r1b = r1[:, None, :].to_broadcast((128, 2, C))
                sq = sml.tile([128, 2, C], F32, tag="sq")
                nc.vector.tensor_tensor(out=sq[:], in0=t2[:], in1=r1b,
                                        op=mybir.AluOpType.mult)
                nc.vector.tensor_tensor(out=sq[:], in0=sq[:], in1=r1b,
                                        op=mybir.AluOpType.mult)
                rt = sml.tile([128, 2, C], F32, tag="rt")
                nc.scalar.activation(out=rt[:], in_=sq[:],
                                     func=mybir.ActivationFunctionType.Sqrt)
                nc.vector.tensor_scalar_add(sq[:], sq[:], 1.0)
                rd = sml.tile([128, 2, C], F32, tag="rd")
                nc.vector.reciprocal(out=rd[:], in_=sq[:])
                g_f = sml.tile([128, 2, C], F32, tag="g_f")
                nc.vector.tensor_tensor(out=g_f[:], in0=rt[:], in1=rd[:],
                                        op=mybir.AluOpType.mult)
                nc.vector.tensor_tensor(out=g_f[:], in0=g_f[:], in1=r1b,
                                        op=mybir.AluOpType.mult)
                v_sb = sml.tile([128, 2, CO], F32, tag="v_sb")
                nc.vector.tensor_tensor(
                    out=v_sb[:].rearrange("p h (c o) -> p h c o", o=O),
                    in0=s_sb[:].rearrange("p h (c o) -> p h c o", o=O),
                    in1=g_f[:, :, :, None].to_broadcast((128, 2, C, O)),
                    op=mybir.AluOpType.mult)

                # ---- G = x^T v, then a = (1/B) sum_{i,o} W*G ----
                for t in range(NT):
                    G_ps = ps_g.tile([128, CO], F32, tag="G_ps")
                    for h in range(2):
                        nc.tensor.matmul(
                            G_ps[:],
                            lhsT=xb_sb[:, h, 128 * t:128 * t + 128],
                            rhs=v_sb[:, h, :],
                            start=(h == 0), stop=(h == 1))
                    nc.scalar.copy(out=Gbig[:, t, :], in_=G_ps[:])
                for ch in range(3):
                    tsl = slice(6 * ch, 6 * ch + 6)
                    nc.vector.tensor_tensor(out=Pp[:, tsl, :], in0=Gbig[:, tsl, :],
                                            in1=W_sb[:, tsl, :],
                                            op=mybir.AluOpType.mult)
                    nc.vector.tensor_reduce(
                        out=P2[:, tsl, :],
                        in_=Pp[:, tsl, :].rearrange("p t (c o) -> p t c o", o=O),
                        axis=mybir.AxisListType.X, op=mybir.AluOpType.add)
                a_dram = dram.tile([RL, 10], F32, tag="a_dram")
                a_stage = sml.tile([16, NT, 10], F32, tag="a_stage")
                for g in range(3):
                    tw = 8 if g < 2 else 2
                    ap_ps = ps_m.tile([16, 80], F32, tag="ap_ps")
                    nc.tensor.matmul(
                        ap_ps[:, :10 * tw],
                        lhsT=oblk_sb[:],
                        rhs=P2[:, 8 * g:8 * g + tw, :].rearrange("p t c -> p (t c)"),
                        start=True, stop=True)
                    nc.scalar.copy(
                        out=a_stage[:, 8 * g:8 * g + tw, :],
                        in_=ap_ps[:, :10 * tw].rearrange("m (t c) -> m t c", c=10))
                nc.sync.dma_start(
                    out=a_dram[:].rearrange("(t m) c -> m t c", m=16),
                    in_=a_stage[:])
                a_sb = sml.tile([96, 3, 10], F32, tag="a_sb")
                nc.sync.dma_start(out=a_sb[:],
                                  in_=a_dram[:].rearrange("(t p) c -> p t c", p=96))
                nc.vector.tensor_tensor(out=b_sb[:], in0=b_sb[:], in1=a_sb[:],
                                        op=mybir.AluOpType.add)

    nc.compile()
    return nc


def _get_nc():
    if _CACHE["nc"] is None:
        _CACHE["nc"] = _build()
    return _CACHE["nc"]


def _row_major(a4):
    """[T, j(16), i(8), ...] -> [T, i, j, ...] flattened rows: row = i*16+j."""
    return np.ascontiguousarray(a4.transpose(0, 2, 1, *range(3, a4.ndim)))


def kernel(x, W):
    global LAST_RESULT
    x = np.ascontiguousarray(np.asarray(x), dtype=np.float32)
    W = np.ascontiguousarray(np.asarray(W), dtype=np.float32)
    assert x.shape == (B, R, I) and W.shape == (R, C, O, I)

    nc = _get_nc()

    # [(r i), (c o)] with i-major row order within each 128-row tile
    Wp = W.transpose(0, 3, 1, 2).reshape(R // 16, 16, I, CO)
    Wp = _row_major(Wp).reshape(R * I, CO)
    oblk = np.zeros((128, 16), np.float32)
    for k in range(128):
        oblk[k, k % 16] = 1.0 / B      # row k = i*16+j -> j = k % 16
    o96 = np.ones((96, 1), np.float32)
    o1 = np.ones((1, 128), np.float32)

    in_maps = []
    for cid in range(NCORE):
        xs = x[:, cid * RL:(cid + 1) * RL, :]                     # [B, 288, 8]
        xT = xs.transpose(1, 2, 0).reshape(NT, 16, I, B)          # [T, j, i, B]
        xT = _row_major(xT).reshape(KI, B)
        xbv = xs.reshape(B, NT, 16, I).transpose(0, 1, 3, 2)      # [B, T, i, j]
        xbv = np.ascontiguousarray(xbv).reshape(B, KI)
        in_maps.append({
            "xT": xT,
            "xb": xbv,
            "Wl": np.ascontiguousarray(Wp[cid * KI:(cid + 1) * KI]),
            "ones_blk": oblk,
            "ones96": o96,
            "ones1": o1,
        })

    res = bass_utils.run_bass_kernel_spmd(
        nc, in_maps, core_ids=list(range(NCORE)),
        trace=bool(os.environ.get("DIGITCAPS_TRACE")))
    LAST_RESULT = res

    s2 = np.zeros((B, CO), np.float64)
    sig = np.zeros((10,), np.float64)
    for cid in range(NCORE):
        s2 += res.results[cid]["sout"]
        sig += res.results[cid]["sigout"][0, :10]
    s3 = (s2.reshape(B, C, O) / sig[None, :, None]).astype(np.float32)
    sq = (s3 * s3).sum(axis=2, keepdims=True)
    v = s3 * (np.sqrt(sq) / (1.0 + sq))
    return v[..., None].astype(np.float32)


# revision 10
# speedup vs baseline: 1.0631x; 1.0631x over previous
"""DigitCaps dynamic-routing kernel for 8 Trainium2 NeuronCores.

Decomposition (validated against the jax reference in a numpy mock):
  u_hat[b,r,c,o] = sum_i W[r,c,o,i] x[b,r,i]   (never materialized)
  s[b,c,o]   = sum_{r,i} x[b,r,i] * cexp[r,c] * W[r,c,o,i] / sigma[c]
  a[r,c]     = (1/B) sum_{i,o} W[r,c,o,i] * G[r,i,c,o],
               G[(r,i),(c,o)] = sum_b x[b,r,i] v[b,c,o]
  softmax over r is computed unnormalized (cexp = exp(b), safe: |b| < 1);
  the divide by sigma[c] = sum_r exp(b[r,c]) folds into the squash.

Sharding: 8-way over routes R (288 each). Each core holds its W shard and
the full batch of x for its routes, so `a` needs no collective. The only
cross-core exchange is the per-iteration AllReduce of the partial
s[b,(c,o)] (plus the 10 local sigma values in the same payload). The final
iteration's s-partials are returned per core; the host sums them and
applies the (tiny) squash while unsharding.

SBUF row space per core: 18 tiles of 128 rows; tile t row (i*16+j) holds
(r = 288*core + 16*t + j, i) — i-major so the exp(b) -> row expansion DMA
writes contiguous partition blocks.
"""
import os
import sys

import numpy as np

_REPO = "/opt/trn_rl_repo"
if _REPO not in sys.path:
    sys.path.insert(0, _REPO)

import concourse.bass as bass  # noqa: E402,F401
import concourse.tile as tile  # noqa: E402
from concourse import bacc, mybir  # noqa: E402
from concourse import bass_utils  # noqa: E402

B, R, C, O, I = 256, 2304, 10, 16, 8
NCORE = 8
RL = R // NCORE          # 288 routes per core
KI = RL * I              # 2304 (r,i) rows per core
NT = KI // 128           # 18 tiles of 128 (r,i) rows
CO = C * O               # 160
NITER = 3
F32 = mybir.dt.float32
SIG_OFF = B * CO         # float offset of the sigma slot in the AR payload
PAY = SIG_OFF + 16       # AR payload floats (s parts + sigma + pad)

_CACHE = {"nc": None}
LAST_RESULT = None


def _build():
    nc = bacc.Bacc("TRN2", target_bir_lowering=False, debug=False, num_devices=NCORE)

    xT_d = nc.dram_tensor("xT", [KI, B], F32, kind="ExternalInput")
    xb_d = nc.dram_tensor("xb", [B, KI], F32, kind="ExternalInput")
    W_d = nc.dram_tensor("Wl", [KI, CO], F32, kind="ExternalInput")
    oblk_d = nc.dram_tensor("ones_blk", [128, 16], F32, kind="ExternalInput")
    o16_d = nc.dram_tensor("ones16", [16, 1], F32, kind="ExternalInput")
    rep16_d = nc.dram_tensor("rep16", [16, 128], F32, kind="ExternalInput")
    o1_d = nc.dram_tensor("ones1", [1, 128], F32, kind="ExternalInput")
    sout_d = nc.dram_tensor("sout", [B, CO], F32, kind="ExternalOutput")
    sigout_d = nc.dram_tensor("sigout", [1, 16], F32, kind="ExternalOutput")

    with tile.TileContext(nc) as tc:
        with tc.tile_pool(name="big", bufs=1) as big, \
             tc.tile_pool(name="sml", bufs=2) as sml, \
             tc.tile_pool(name="ps_s", bufs=2, space="PSUM") as ps_s, \
             tc.tile_pool(name="ps_g", bufs=2, space="PSUM") as ps_g, \
             tc.tile_pool(name="ps_m", bufs=1, space="PSUM") as ps_m, \
             tc.tile_pool(name="dram", bufs=2, space="DRAM") as dram:

            # ---- warmup AllReduce: absorbs cross-core launch skew and the
            # collective machinery's first-call cost while inputs stream in
            wu_sb = sml.tile([1, 16], F32, tag="wu")
            nc.vector.memset(wu_sb[:], 1.0)
            wu_in = dram.tile([16], F32, tag="wu_in")
            wu_out = dram.tile([16], F32, tag="wu_out", addr_space="Shared")
            nc.sync.dma_start(out=wu_in[:].rearrange("(p f) -> p f", p=1), in_=wu_sb[:])
            nc.gpsimd.collective_compute(
                "AllReduce", mybir.AluOpType.add,
                replica_groups=[list(range(NCORE))],
                ins=[wu_in.opt()], outs=[wu_out.opt()])
            nc.sync.dma_start(
                out=sigout_d.ap()[:, 12:16],
                in_=wu_out[0:4].rearrange("(p f) -> p f", p=1))

            # ---- preload (chunked so the first matmuls start early) ----
            W_sb = big.tile([128, NT, CO], F32, tag="W")
            xT_sb = big.tile([128, NT, B], F32, tag="xT")
            for ch in range(3):
                tsl = slice(6 * ch, 6 * ch + 6)
                nc.sync.dma_start(
                    out=W_sb[:, tsl, :],
                    in_=W_d.ap().rearrange("(t p) f -> p t f", p=128)[:, tsl, :])
                nc.sync.dma_start(
                    out=xT_sb[:, tsl, :],
                    in_=xT_d.ap().rearrange("(t p) b -> p t b", p=128)[:, tsl, :])
            xb_sb = big.tile([128, 2, KI], F32, tag="xb")
            nc.sync.dma_start(out=xb_sb[:], in_=xb_d.ap().rearrange("(h p) k -> p h k", p=128))
            oblk_sb = big.tile([128, 16], F32, tag="oblk")
            nc.sync.dma_start(out=oblk_sb[:], in_=oblk_d.ap())
            o16_sb = big.tile([16, 1], F32, tag="o16")
            nc.sync.dma_start(out=o16_sb[:], in_=o16_d.ap())
            rep16_sb = big.tile([16, 128], F32, tag="rep16")
            nc.sync.dma_start(out=rep16_sb[:], in_=rep16_d.ap())
            o1_sb = big.tile([1, 128], F32, tag="o1")
            nc.sync.dma_start(out=o1_sb[:], in_=o1_d.ap())

            # routing state b lives in the a-pieces layout: [16, T, c],
            # partition m + tile column T <-> local route r = 16*T + m
            b_sb = big.tile([16, NT, C], F32, tag="b_state")
            nc.vector.memset(b_sb[:], 0.0)

            M_sb = big.tile([128, NT, CO], F32, tag="M")
            Gbig = big.tile([128, NT, CO], F32, tag="G")
            Pp = big.tile([128, NT, CO], F32, tag="Pp")
            P2 = big.tile([128, NT, C], F32, tag="P2")
            cE = big.tile([128, NT, C], F32, tag="cE")

            for it in range(NITER):
                # ---- unnormalized softmax from b state (iters >= 1) ----
                if it > 0:
                    exp_sb = sml.tile([16, NT, C], F32, tag="exp")
                    nc.scalar.activation(out=exp_sb[:], in_=b_sb[:],
                                         func=mybir.ActivationFunctionType.Exp)
                    sig_ps = ps_m.tile([1, NT * C], F32, tag="sigps")
                    nc.tensor.matmul(sig_ps[:], lhsT=o16_sb[:],
                                     rhs=exp_sb[:].rearrange("p t c -> p (t c)"),
                                     start=True, stop=True)
                    sig_sb = sml.tile([1, 10], F32, tag="sigloc")
                    nc.vector.tensor_reduce(
                        out=sig_sb[:],
                        in_=sig_ps[:].rearrange("p (t c) -> p c t", c=10),
                        axis=mybir.AxisListType.X, op=mybir.AluOpType.add)
                    # expand rows: cE[16*i + j, t, c] = exp[j, t, c] via one
                    # constant replication matmul on the PE
                    cE_ps = ps_m.tile([128, NT * C], F32, tag="cE_ps")
                    nc.tensor.matmul(cE_ps[:], lhsT=rep16_sb[:],
                                     rhs=exp_sb[:].rearrange("p t c -> p (t c)"),
                                     start=True, stop=True)
                    nc.scalar.copy(out=cE[:].rearrange("p t c -> p (t c)"),
                                   in_=cE_ps[:])
                    # M = W * cE (broadcast over o), chunked for PE overlap
                    for ch in range(3):
                        tsl = slice(6 * ch, 6 * ch + 6)
                        nc.vector.tensor_tensor(
                            out=M_sb[:, tsl, :].rearrange("p t (c o) -> p t c o", o=O),
                            in0=W_sb[:, tsl, :].rearrange("p t (c o) -> p t c o", o=O),
                            in1=cE[:, tsl, :, None].to_broadcast((128, 6, C, O)),
                            op=mybir.AluOpType.mult)
                else:
                    sig_sb = sml.tile([1, 10], F32, tag="sigloc")
                    nc.vector.memset(sig_sb[:], float(RL))

                # ---- s partial matmuls ----
                rhs_src = W_sb if it == 0 else M_sb
                psum_s = [ps_s.tile([128, CO], F32, tag="psum_s", name=f"psum_s{_h}") for _h in range(2)]
                for h in range(2):
                    for t in range(NT):
                        nc.tensor.matmul(
                            psum_s[h][:],
                            lhsT=xT_sb[:, t, 128 * h:128 * h + 128],
                            rhs=rhs_src[:, t, :],
                            start=(t == 0), stop=(t == NT - 1))

                sp_sb = sml.tile([128, 2, CO], F32, tag="sp_sb")
                for h in range(2):
                    nc.scalar.copy(out=sp_sb[:, h, :], in_=psum_s[h][:])

                if it == NITER - 1:
                    nc.sync.dma_start(
                        out=sout_d.ap().rearrange("(h p) f -> p h f", p=128),
                        in_=sp_sb[:])
                    nc.sync.dma_start(out=sigout_d.ap()[:, 0:10], in_=sig_sb[:])
                    break

                # ---- AllReduce payload: s parts + sigma ----
                ar_in = dram.tile([PAY], F32, tag="ar_in")
                ar_out = dram.tile([PAY], F32, tag="ar_out", addr_space="Shared")
                nc.sync.dma_start(
                    out=ar_in[0:B * CO].rearrange("(h p f) -> p h f", p=128, f=CO),
                    in_=sp_sb[:])
                nc.sync.dma_start(
                    out=ar_in[SIG_OFF:SIG_OFF + 10].rearrange("(p f) -> p f", p=1),
                    in_=sig_sb[:])
                nc.gpsimd.collective_compute(
                    "AllReduce", mybir.AluOpType.add,
                    replica_groups=[list(range(NCORE))],
                    ins=[ar_in.opt()], outs=[ar_out.opt()])

                s_sb = sml.tile([128, 2, CO], F32, tag="s_sb")
                nc.sync.dma_start(
                    out=s_sb[:],
                    in_=ar_out[0:B * CO].rearrange("(h p f) -> p h f", p=128, f=CO))
                sig_all = sml.tile([1, 10], F32, tag="sig_all")
                nc.sync.dma_start(
                    out=sig_all[:],
                    in_=ar_out[SIG_OFF:SIG_OFF + 10].rearrange("(p f) -> p f", p=1))

                # ---- squash: v = s_u * sqrt(sq)/(1+sq) / sigma,
                #      sq = (sum_o s_u^2)/sigma^2 ----
                sigB_ps = ps_m.tile([128, 10], F32, tag="sigB")
                nc.tensor.matmul(sigB_ps[:], lhsT=o1_sb[:], rhs=sig_all[:],
                                 start=True, stop=True)
                r1 = sml.tile([128, 10], F32, tag="r1")
                nc.vector.reciprocal(out=r1[:], in_=sigB_ps[:])
                ssq = sml.tile([128, 2, CO], F32, tag="ssq")
                nc.vector.tensor_tensor(out=ssq[:], in0=s_sb[:], in1=s_sb[:],
                                        op=mybir.AluOpType.mult)
                t2 = sml.tile([128, 2, C], F32, tag="t2")
                nc.vector.tensor_reduce(
                    out=t2[:], in_=ssq[:].rearrange("p h (c o) -> p h c o", o=O),
                    axis=mybir.AxisListType.X, op=mybir.AluOpType.add)
                r1b = r1[:, None, :].to_broadcast((128, 2, C))
                sq = sml.tile([128, 2, C], F32, tag="sq")
                nc.vector.tensor_tensor(out=sq[:], in0=t2[:], in1=r1b,
                                        op=mybir.AluOpType.mult)
                nc.vector.tensor_tensor(out=sq[:], in0=sq[:], in1=r1b,
                                        op=mybir.AluOpType.mult)
                rt = sml.tile([128, 2, C], F32, tag="rt")
                nc.scalar.activation(out=rt[:], in_=sq[:],
                                     func=mybir.ActivationFunctionType.Sqrt)
                nc.vector.tensor_scalar_add(sq[:], sq[:], 1.0)
                rd = sml.tile([128, 2, C], F32, tag="rd")
                nc.vector.reciprocal(out=rd[:], in_=sq[:])
                g_f = sml.tile([128, 2, C], F32, tag="g_f")
                nc.vector.tensor_tensor(out=g_f[:], in0=rt[:], in1=rd[:],
                                        op=mybir.AluOpType.mult)
                nc.vector.tensor_tensor(out=g_f[:], in0=g_f[:], in1=r1b,
                                        op=mybir.AluOpType.mult)
                v_sb = sml.tile([128, 2, CO], F32, tag="v_sb")
                nc.vector.tensor_tensor(
                    out=v_sb[:].rearrange("p h (c o) -> p h c o", o=O),
                    in0=s_sb[:].rearrange("p h (c o) -> p h c o", o=O),
                    in1=g_f[:, :, :, None].to_broadcast((128, 2, C, O)),
                    op=mybir.AluOpType.mult)

                # ---- G = x^T v, then a = (1/B) sum_{i,o} W*G ----
                for t in range(NT):
                    G_ps = ps_g.tile([128, CO], F32, tag="G_ps")
                    for h in range(2):
                        nc.tensor.matmul(
                            G_ps[:],
                            lhsT=xb_sb[:, h, 128 * t:128 * t + 128],
                            rhs=v_sb[:, h, :],
                            start=(h == 0), stop=(h == 1))
                    nc.scalar.copy(out=Gbig[:, t, :], in_=G_ps[:])
                for ch in range(3):
                    tsl = slice(6 * ch, 6 * ch + 6)
                    nc.vector.tensor_tensor(out=Pp[:, tsl, :], in0=Gbig[:, tsl, :],
                                            in1=W_sb[:, tsl, :],
                                            op=mybir.AluOpType.mult)
                    nc.vector.tensor_reduce(
                        out=P2[:, tsl, :],
                        in_=Pp[:, tsl, :].rearrange("p t (c o) -> p t c o", o=O),
                        axis=mybir.AxisListType.X, op=mybir.AluOpType.add)
                a_stage = sml.tile([16, NT, C], F32, tag="a_stage")
                for g in range(3):
                    tw = 8 if g < 2 else 2
                    ap_ps = ps_m.tile([16, 80], F32, tag="ap_ps")
                    nc.tensor.matmul(
                        ap_ps[:, :10 * tw],
                        lhsT=oblk_sb[:],
                        rhs=P2[:, 8 * g:8 * g + tw, :].rearrange("p t c -> p (t c)"),
                        start=True, stop=True)
                    nc.scalar.copy(
                        out=a_stage[:, 8 * g:8 * g + tw, :],
                        in_=ap_ps[:, :10 * tw].rearrange("m (t c) -> m t c", c=10))
                nc.vector.tensor_tensor(out=b_sb[:], in0=b_sb[:], in1=a_stage[:],
                                        op=mybir.AluOpType.add)

    nc.compile()
    return nc


def _get_nc():
    if _CACHE["nc"] is None:
        _CACHE["nc"] = _build()
    return _CACHE["nc"]


def _row_major(a4):
    """[T, j(16), i(8), ...] -> [T, i, j, ...] flattened rows: row = i*16+j."""
    return np.ascontiguousarray(a4.transpose(0, 2, 1, *range(3, a4.ndim)))


def kernel(x, W):
    global LAST_RESULT
    x = np.ascontiguousarray(np.asarray(x), dtype=np.float32)
    W = np.ascontiguousarray(np.asarray(W), dtype=np.float32)
    assert x.shape == (B, R, I) and W.shape == (R, C, O, I)

    nc = _get_nc()

    # [(r i), (c o)] with i-major row order within each 128-row tile
    Wp = W.transpose(0, 3, 1, 2).reshape(R // 16, 16, I, CO)
    Wp = _row_major(Wp).reshape(R * I, CO)
    oblk = np.zeros((128, 16), np.float32)
    for k in range(128):
        oblk[k, k % 16] = 1.0 / B      # row k = i*16+j -> j = k % 16
    o16 = np.ones((16, 1), np.float32)
    rep16 = np.zeros((16, 128), np.float32)
    for p in range(128):
        rep16[p % 16, p] = 1.0
    o1 = np.ones((1, 128), np.float32)

    in_maps = []
    for cid in range(NCORE):
        xs = x[:, cid * RL:(cid + 1) * RL, :]                     # [B, 288, 8]
        xT = xs.transpose(1, 2, 0).reshape(NT, 16, I, B)          # [T, j, i, B]
        xT = _row_major(xT).reshape(KI, B)
        xbv = xs.reshape(B, NT, 16, I).transpose(0, 1, 3, 2)      # [B, T, i, j]
        xbv = np.ascontiguousarray(xbv).reshape(B, KI)
        in_maps.append({
            "xT": xT,
            "xb": xbv,
            "Wl": np.ascontiguousarray(Wp[cid * KI:(cid + 1) * KI]),
            "ones_blk": oblk,
            "ones16": o16,
            "rep16": rep16,
            "ones1": o1,
        })

    res = bass_utils.run_bass_kernel_spmd(
        nc, in_maps, core_ids=list(range(NCORE)),
        trace=bool(os.environ.get("DIGITCAPS_TRACE")))
    LAST_RESULT = res

    s2 = np.zeros((B, CO), np.float64)
    sig = np.zeros((10,), np.float64)
    for cid in range(NCORE):
        s2 += res.results[cid]["sout"]
        sig += res.results[cid]["sigout"][0, :10]
    s3 = (s2.reshape(B, C, O) / sig[None, :, None]).astype(np.float32)
    sq = (s3 * s3).sum(axis=2, keepdims=True)
    v = s3 * (np.sqrt(sq) / (1.0 + sq))
    return v[..., None].astype(np.float32)


# revision 13
# speedup vs baseline: 1.1354x; 1.0680x over previous
"""DigitCaps dynamic-routing kernel for 8 Trainium2 NeuronCores.

Decomposition (validated against the jax reference in a numpy mock):
  u_hat[b,r,c,o] = sum_i W[r,c,o,i] x[b,r,i]   (never materialized)
  s[b,c,o]   = sum_{r,i} x[b,r,i] * cexp[r,c] * W[r,c,o,i] / sigma[c]
  a[r,c]     = (1/B) sum_{i,o} W[r,c,o,i] * G[r,i,c,o],
               G[(r,i),(c,o)] = sum_b x[b,r,i] v[b,c,o]
  softmax over r is computed unnormalized (cexp = exp(b), safe: |b| < 1);
  the divide by sigma[c] = sum_r exp(b[r,c]) folds into the squash.

Sharding: 8-way over routes R (288 each). Each core holds its W shard and
the full batch of x for its routes, so `a` needs no collective. The only
cross-core exchange is the per-iteration AllReduce of the partial
s[b,(c,o)] (plus the 10 local sigma values in the same payload). The final
iteration's s-partials are returned per core; the host sums them and
applies the (tiny) squash while unsharding.

SBUF row space per core: 18 tiles of 128 rows; tile t row (i*16+j) holds
(r = 288*core + 16*t + j, i) — i-major so the exp(b) -> row expansion DMA
writes contiguous partition blocks.
"""
import os
import sys

import numpy as np

_REPO = "/opt/trn_rl_repo"
if _REPO not in sys.path:
    sys.path.insert(0, _REPO)

import concourse.bass as bass  # noqa: E402,F401
import concourse.tile as tile  # noqa: E402
from concourse import bacc, mybir  # noqa: E402
from concourse import bass_utils  # noqa: E402

B, R, C, O, I = 256, 2304, 10, 16, 8
NCORE = 8
RL = R // NCORE          # 288 routes per core
KI = RL * I              # 2304 (r,i) rows per core
NT = KI // 128           # 18 tiles of 128 (r,i) rows
CO = C * O               # 160
NITER = 3
F32 = mybir.dt.float32
SIG_OFF = B * CO         # float offset of the sigma slot in the AR payload
PAY = SIG_OFF + 16       # AR payload floats (s parts + sigma + pad)

_CACHE = {"nc": None}
LAST_RESULT = None


def _build():
    nc = bacc.Bacc("TRN2", target_bir_lowering=False, debug=False, num_devices=NCORE)

    xT_d = nc.dram_tensor("xT", [KI, B], F32, kind="ExternalInput")
    xb_d = nc.dram_tensor("xb", [B, KI], F32, kind="ExternalInput")
    W_d = nc.dram_tensor("Wl", [KI, CO], F32, kind="ExternalInput")
    oblk_d = nc.dram_tensor("ones_blk", [128, 16], F32, kind="ExternalInput")
    o16_d = nc.dram_tensor("ones16", [16, 1], F32, kind="ExternalInput")
    rep16_d = nc.dram_tensor("rep16", [16, 128], F32, kind="ExternalInput")
    o1_d = nc.dram_tensor("ones1", [1, 128], F32, kind="ExternalInput")
    sout_d = nc.dram_tensor("sout", [B, CO], F32, kind="ExternalOutput")
    sigout_d = nc.dram_tensor("sigout", [1, 16], F32, kind="ExternalOutput")

    with tile.TileContext(nc) as tc:
        with tc.tile_pool(name="big", bufs=1) as big, \
             tc.tile_pool(name="sml", bufs=2) as sml, \
             tc.tile_pool(name="ps_s", bufs=2, space="PSUM") as ps_s, \
             tc.tile_pool(name="ps_g", bufs=4, space="PSUM") as ps_g, \
             tc.tile_pool(name="ps_m", bufs=1, space="PSUM") as ps_m, \
             tc.tile_pool(name="dram", bufs=2, space="DRAM") as dram:

            # ---- preload (chunked so the first matmuls start early) ----
            W_sb = big.tile([128, NT, CO], F32, tag="W")
            xT_sb = big.tile([128, NT, B], F32, tag="xT")
            for ch in range(3):
                tsl = slice(6 * ch, 6 * ch + 6)
                nc.sync.dma_start(
                    out=W_sb[:, tsl, :],
                    in_=W_d.ap().rearrange("(t p) f -> p t f", p=128)[:, tsl, :])
                nc.sync.dma_start(
                    out=xT_sb[:, tsl, :],
                    in_=xT_d.ap().rearrange("(t p) b -> p t b", p=128)[:, tsl, :])
            xb_sb = big.tile([128, 2, KI], F32, tag="xb")
            xb_dma = None   # deferred until the first s0 matmul has its data
            oblk_sb = big.tile([128, 16], F32, tag="oblk")
            nc.sync.dma_start(out=oblk_sb[:], in_=oblk_d.ap())
            o16_sb = big.tile([16, 1], F32, tag="o16")
            nc.sync.dma_start(out=o16_sb[:], in_=o16_d.ap())
            rep16_sb = big.tile([16, 128], F32, tag="rep16")
            nc.sync.dma_start(out=rep16_sb[:], in_=rep16_d.ap())
            o1_sb = big.tile([1, 128], F32, tag="o1")
            nc.sync.dma_start(out=o1_sb[:], in_=o1_d.ap())

            # routing state b lives in the a-pieces layout: [16, T, c],
            # partition m + tile column T <-> local route r = 16*T + m
            b_sb = big.tile([16, NT, C], F32, tag="b_state")
            nc.vector.memset(b_sb[:], 0.0)

            M_sb = big.tile([128, NT, CO], F32, tag="M")
            BF16 = mybir.dt.bfloat16
            Wbf = big.tile([128, NT, CO], BF16, tag="Wbf")
            wbf_cvt = nc.scalar.copy(out=Wbf[:], in_=W_sb[:])
            Gbig = big.tile([128, NT, CO], BF16, tag="G")
            Pp = big.tile([128, NT, CO], BF16, tag="Pp")
            P2 = big.tile([128, NT, C], F32, tag="P2")
            cE = big.tile([128, NT, C], F32, tag="cE")

            for it in range(NITER):
                # ---- unnormalized softmax from b state (iters >= 1) ----
                if it > 0:
                    exp_sb = sml.tile([16, NT, C], F32, tag="exp")
                    nc.scalar.activation(out=exp_sb[:], in_=b_sb[:],
                                         func=mybir.ActivationFunctionType.Exp)
                    sig_ps = ps_m.tile([128, NT * C], F32, tag="ps_small", name="sig_ps")[0:1]
                    nc.tensor.matmul(sig_ps[:], lhsT=o16_sb[:],
                                     rhs=exp_sb[:].rearrange("p t c -> p (t c)"),
                                     start=True, stop=True)
                    sig_sb = sml.tile([1, 10], F32, tag="sigloc")
                    nc.vector.tensor_reduce(
                        out=sig_sb[:],
                        in_=sig_ps[:].rearrange("p (t c) -> p c t", c=10),
                        axis=mybir.AxisListType.X, op=mybir.AluOpType.add)
                    # expand rows: cE[16*i + j, t, c] = exp[j, t, c] via one
                    # constant replication matmul on the PE
                    cE_ps = ps_m.tile([128, NT * C], F32, tag="ps_mid", name="cE_ps")
                    nc.tensor.matmul(cE_ps[:], lhsT=rep16_sb[:],
                                     rhs=exp_sb[:].rearrange("p t c -> p (t c)"),
                                     start=True, stop=True)
                    nc.scalar.copy(out=cE[:].rearrange("p t c -> p (t c)"),
                                   in_=cE_ps[:])
                    # M = W * cE (broadcast over o), chunked for PE overlap
                    for ch in range(3):
                        tsl = slice(6 * ch, 6 * ch + 6)
                        nc.vector.tensor_tensor(
                            out=M_sb[:, tsl, :].rearrange("p t (c o) -> p t c o", o=O),
                            in0=W_sb[:, tsl, :].rearrange("p t (c o) -> p t c o", o=O),
                            in1=cE[:, tsl, :, None].to_broadcast((128, 6, C, O)),
                            op=mybir.AluOpType.mult)
                else:
                    sig_sb = sml.tile([1, 10], F32, tag="sigloc")
                    nc.vector.memset(sig_sb[:], float(RL))

                # ---- s partial matmuls ----
                rhs_src = W_sb if it == 0 else M_sb
                psum_s = [ps_s.tile([128, CO], F32, tag="psum_s", name=f"psum_s{_h}") for _h in range(2)]
                for h in range(2):
                    for t in range(NT):
                        mm = nc.tensor.matmul(
                            psum_s[h][:],
                            lhsT=xT_sb[:, t, 128 * h:128 * h + 128],
                            rhs=rhs_src[:, t, :],
                            start=(t == 0), stop=(t == NT - 1))
                        if it == 0 and h == 0 and t == 0:
                            xb_dma = nc.sync.dma_start(
                                out=xb_sb[:],
                                in_=xb_d.ap().rearrange("(h p) k -> p h k", p=128))
                            tile.add_dep_helper(
                                xb_dma.ins, mm.ins,
                                reason="defer xb load until W/xT loads are done")

                sp_sb = sml.tile([128, 2, CO], F32, tag="sp_sb")
                for h in range(2):
                    nc.scalar.copy(out=sp_sb[:, h, :], in_=psum_s[h][:])

                if it == NITER - 1:
                    nc.sync.dma_start(
                        out=sout_d.ap().rearrange("(h p) f -> p h f", p=128),
                        in_=sp_sb[:])
                    nc.sync.dma_start(out=sigout_d.ap()[:, 0:10], in_=sig_sb[:])
                    break

                # ---- AllReduce payload: s parts + sigma ----
                ar_in = dram.tile([PAY], F32, tag="ar_in")
                ar_out = dram.tile([PAY], F32, tag="ar_out", addr_space="Shared")
                nc.sync.dma_start(
                    out=ar_in[0:B * CO].rearrange("(h p f) -> p h f", p=128, f=CO),
                    in_=sp_sb[:])
                nc.sync.dma_start(
                    out=ar_in[SIG_OFF:SIG_OFF + 10].rearrange("(p f) -> p f", p=1),
                    in_=sig_sb[:])
                nc.gpsimd.collective_compute(
                    "AllReduce", mybir.AluOpType.add,
                    replica_groups=[list(range(NCORE))],
                    ins=[ar_in.opt()], outs=[ar_out.opt()])

                s_sb = sml.tile([128, 2, CO], F32, tag="s_sb")
                nc.sync.dma_start(
                    out=s_sb[:],
                    in_=ar_out[0:B * CO].rearrange("(h p f) -> p h f", p=128, f=CO))
                sig_all = sml.tile([1, 10], F32, tag="sig_all")
                nc.sync.dma_start(
                    out=sig_all[:],
                    in_=ar_out[SIG_OFF:SIG_OFF + 10].rearrange("(p f) -> p f", p=1))

                # ---- squash: v = s_u * sqrt(sq)/(1+sq) / sigma,
                #      sq = (sum_o s_u^2)/sigma^2 ----
                sigB_ps = ps_m.tile([128, NT * C], F32, tag="ps_mid", name="sigB_ps")[:, 0:10]
                nc.tensor.matmul(sigB_ps[:], lhsT=o1_sb[:], rhs=sig_all[:],
                                 start=True, stop=True)
                sigB = sml.tile([128, 10], F32, tag="sigB")
                nc.scalar.copy(out=sigB[:], in_=sigB_ps[:])
                sig2 = sml.tile([128, 10], F32, tag="sig2")
                nc.vector.tensor_tensor(out=sig2[:], in0=sigB[:], in1=sigB[:],
                                        op=mybir.AluOpType.mult)
                ssq = sml.tile([128, 2, CO], F32, tag="ssq")
                nc.vector.tensor_tensor(out=ssq[:], in0=s_sb[:], in1=s_sb[:],
                                        op=mybir.AluOpType.mult)
                t2 = sml.tile([128, 2, C], F32, tag="t2")
                nc.vector.tensor_reduce(
                    out=t2[:], in_=ssq[:].rearrange("p h (c o) -> p h c o", o=O),
                    axis=mybir.AxisListType.X, op=mybir.AluOpType.add)
                # g = sqrt(t2) / (sigma^2 + t2); then v = s_u * g
                rt = sml.tile([128, 2, C], F32, tag="rt")
                nc.scalar.activation(out=rt[:], in_=t2[:],
                                     func=mybir.ActivationFunctionType.Sqrt)
                dn = sml.tile([128, 2, C], F32, tag="dn")
                nc.vector.tensor_tensor(
                    out=dn[:], in0=t2[:],
                    in1=sig2[:, None, :].to_broadcast((128, 2, C)),
                    op=mybir.AluOpType.add)
                nc.vector.reciprocal(out=dn[:], in_=dn[:])
                g_f = sml.tile([128, 2, C], F32, tag="g_f")
                nc.vector.tensor_tensor(out=g_f[:], in0=rt[:], in1=dn[:],
                                        op=mybir.AluOpType.mult)
                v_sb = sml.tile([128, 2, CO], F32, tag="v_sb")
                nc.vector.tensor_tensor(
                    out=v_sb[:].rearrange("p h (c o) -> p h c o", o=O),
                    in0=s_sb[:].rearrange("p h (c o) -> p h c o", o=O),
                    in1=g_f[:, :, :, None].to_broadcast((128, 2, C, O)),
                    op=mybir.AluOpType.mult)

                # ---- G = x^T v, then a = (1/B) sum_{i,o} W*G ----
                a_stage = sml.tile([16, NT, C], F32, tag="a_stage")
                for ch in range(3):
                    tsl = slice(6 * ch, 6 * ch + 6)
                    for t in range(6 * ch, 6 * ch + 6):
                        G_ps = ps_g.tile([128, CO], F32, tag="G_ps")
                        for h in range(2):
                            nc.tensor.matmul(
                                G_ps[:],
                                lhsT=xb_sb[:, h, 128 * t:128 * t + 128],
                                rhs=v_sb[:, h, :],
                                start=(h == 0), stop=(h == 1))
                        nc.scalar.copy(out=Gbig[:, t, :], in_=G_ps[:])
                    nc.vector.tensor_tensor(out=Pp[:, tsl, :], in0=Gbig[:, tsl, :],
                                            in1=Wbf[:, tsl, :],
                                            op=mybir.AluOpType.mult)
                    nc.vector.tensor_reduce(
                        out=P2[:, tsl, :],
                        in_=Pp[:, tsl, :].rearrange("p t (c o) -> p t c o", o=O),
                        axis=mybir.AxisListType.X, op=mybir.AluOpType.add)
                    ap_ps = ps_m.tile([128, NT * C], F32, tag="ps_small", name="ap_ps")[0:16, 0:60]
                    nc.tensor.matmul(
                        ap_ps[:],
                        lhsT=oblk_sb[:],
                        rhs=P2[:, tsl, :].rearrange("p t c -> p (t c)"),
                        start=True, stop=True)
                    nc.scalar.copy(
                        out=a_stage[:, tsl, :],
                        in_=ap_ps[:].rearrange("m (t c) -> m t c", c=10))
                nc.vector.tensor_tensor(out=b_sb[:], in0=b_sb[:], in1=a_stage[:],
                                        op=mybir.AluOpType.add)

    nc.compile()
    return nc


def _get_nc():
    if _CACHE["nc"] is None:
        _CACHE["nc"] = _build()
    return _CACHE["nc"]


def _row_major(a4):
    """[T, j(16), i(8), ...] -> [T, i, j, ...] flattened rows: row = i*16+j."""
    return np.ascontiguousarray(a4.transpose(0, 2, 1, *range(3, a4.ndim)))


def kernel(x, W):
    global LAST_RESULT
    x = np.ascontiguousarray(np.asarray(x), dtype=np.float32)
    W = np.ascontiguousarray(np.asarray(W), dtype=np.float32)
    assert x.shape == (B, R, I) and W.shape == (R, C, O, I)

    nc = _get_nc()

    # [(r i), (c o)] with i-major row order within each 128-row tile
    Wp = W.transpose(0, 3, 1, 2).reshape(R // 16, 16, I, CO)
    Wp = _row_major(Wp).reshape(R * I, CO)
    oblk = np.zeros((128, 16), np.float32)
    for k in range(128):
        oblk[k, k % 16] = 1.0 / B      # row k = i*16+j -> j = k % 16
    o16 = np.ones((16, 1), np.float32)
    rep16 = np.zeros((16, 128), np.float32)
    for p in range(128):
        rep16[p % 16, p] = 1.0
    o1 = np.ones((1, 128), np.float32)

    in_maps = []
    for cid in range(NCORE):
        xs = x[:, cid * RL:(cid + 1) * RL, :]                     # [B, 288, 8]
        xT = xs.transpose(1, 2, 0).reshape(NT, 16, I, B)          # [T, j, i, B]
        xT = _row_major(xT).reshape(KI, B)
        xbv = xs.reshape(B, NT, 16, I).transpose(0, 1, 3, 2)      # [B, T, i, j]
        xbv = np.ascontiguousarray(xbv).reshape(B, KI)
        in_maps.append({
            "xT": xT,
            "xb": xbv,
            "Wl": np.ascontiguousarray(Wp[cid * KI:(cid + 1) * KI]),
            "ones_blk": oblk,
            "ones16": o16,
            "rep16": rep16,
            "ones1": o1,
        })

    res = bass_utils.run_bass_kernel_spmd(
        nc, in_maps, core_ids=list(range(NCORE)),
        trace=bool(os.environ.get("DIGITCAPS_TRACE")))
    LAST_RESULT = res

    s2 = np.zeros((B, CO), np.float64)
    sig = np.zeros((10,), np.float64)
    for cid in range(NCORE):
        s2 += res.results[cid]["sout"]
        sig += res.results[cid]["sigout"][0, :10]
    s3 = (s2.reshape(B, C, O) / sig[None, :, None]).astype(np.float32)
    sq = (s3 * s3).sum(axis=2, keepdims=True)
    v = s3 * (np.sqrt(sq) / (1.0 + sq))
    return v[..., None].astype(np.float32)


# revision 14
# speedup vs baseline: 1.3805x; 1.2159x over previous
"""DigitCaps dynamic-routing kernel for 8 Trainium2 NeuronCores.

Decomposition (validated against the jax reference in a numpy mock):
  u_hat[b,r,c,o] = sum_i W[r,c,o,i] x[b,r,i]   (never materialized)
  s[b,c,o]   = sum_{r,i} x[b,r,i] * cexp[r,c] * W[r,c,o,i] / sigma[c]
  a[r,c]     = (1/B) sum_{i,o} W[r,c,o,i] * G[r,i,c,o],
               G[(r,i),(c,o)] = sum_b x[b,r,i] v[b,c,o]
  softmax over r is computed unnormalized (cexp = exp(b), safe: |b| < 1);
  the divide by sigma[c] = sum_r exp(b[r,c]) folds into the squash.

Sharding: 8-way over routes R (288 each). Each core holds its W shard and
the full batch of x for its routes, so `a` needs no collective. The only
cross-core exchange is the per-iteration AllReduce of the partial
s[b,(c,o)] (plus the 10 local sigma values in the same payload). The final
iteration's s-partials are returned per core; the host sums them and
applies the (tiny) squash while unsharding.

SBUF row space per core: 18 tiles of 128 rows; tile t row (i*16+j) holds
(r = 288*core + 16*t + j, i) — i-major so the exp(b) -> row expansion DMA
writes contiguous partition blocks.
"""
import os
import sys

import numpy as np

_REPO = "/opt/trn_rl_repo"
if _REPO not in sys.path:
    sys.path.insert(0, _REPO)

import concourse.bass as bass  # noqa: E402,F401
import concourse.tile as tile  # noqa: E402
from concourse import bacc, mybir  # noqa: E402
from concourse import bass_utils  # noqa: E402

B, R, C, O, I = 256, 2304, 10, 16, 8
NCORE = 8
RL = R // NCORE          # 288 routes per core
KI = RL * I              # 2304 (r,i) rows per core
NT = KI // 128           # 18 tiles of 128 (r,i) rows
CO = C * O               # 160
NITER = 3
F32 = mybir.dt.float32
BF16 = mybir.dt.bfloat16
USE_BF16 = not os.environ.get("DIGITCAPS_FP32")
MMDT = BF16 if USE_BF16 else F32
SIG_OFF = B * CO         # float offset of the sigma slot in the AR payload
PAY = SIG_OFF + 16       # AR payload floats (s parts + sigma + pad)

_CACHE = {"nc": None}
LAST_RESULT = None


def _build():
    nc = bacc.Bacc("TRN2", target_bir_lowering=False, debug=False, num_devices=NCORE)

    xT_d = nc.dram_tensor("xT", [KI, B], MMDT, kind="ExternalInput")
    xb_d = nc.dram_tensor("xb", [B, KI], MMDT, kind="ExternalInput")
    W_d = nc.dram_tensor("Wl", [KI, CO], MMDT, kind="ExternalInput")
    oblk_d = nc.dram_tensor("ones_blk", [128, 16], F32, kind="ExternalInput")
    o16_d = nc.dram_tensor("ones16", [16, 1], F32, kind="ExternalInput")
    rep16_d = nc.dram_tensor("rep16", [16, 128], F32, kind="ExternalInput")
    o1_d = nc.dram_tensor("ones1", [1, 128], F32, kind="ExternalInput")
    sout_d = nc.dram_tensor("sout", [B, CO], F32, kind="ExternalOutput")
    sigout_d = nc.dram_tensor("sigout", [1, 16], F32, kind="ExternalOutput")

    with tile.TileContext(nc) as tc:
        with tc.tile_pool(name="big", bufs=1) as big, \
             tc.tile_pool(name="sml", bufs=2) as sml, \
             tc.tile_pool(name="ps_s", bufs=2, space="PSUM") as ps_s, \
             tc.tile_pool(name="ps_g", bufs=4, space="PSUM") as ps_g, \
             tc.tile_pool(name="ps_m", bufs=1, space="PSUM") as ps_m, \
             tc.tile_pool(name="dram", bufs=2, space="DRAM") as dram:

            # ---- preload (chunked so the first matmuls start early) ----
            W_sb = big.tile([128, NT, CO], MMDT, tag="W")
            xT_sb = big.tile([128, NT, B], MMDT, tag="xT")
            for ch in range(3):
                tsl = slice(6 * ch, 6 * ch + 6)
                nc.sync.dma_start(
                    out=W_sb[:, tsl, :],
                    in_=W_d.ap().rearrange("(t p) f -> p t f", p=128)[:, tsl, :])
                nc.sync.dma_start(
                    out=xT_sb[:, tsl, :],
                    in_=xT_d.ap().rearrange("(t p) b -> p t b", p=128)[:, tsl, :])
            xb_sb = big.tile([128, 2, KI], MMDT, tag="xb")
            xb_dma = None   # deferred until the first s0 matmul has its data
            oblk_sb = big.tile([128, 16], F32, tag="oblk")
            nc.sync.dma_start(out=oblk_sb[:], in_=oblk_d.ap())
            o16_sb = big.tile([16, 1], F32, tag="o16")
            nc.sync.dma_start(out=o16_sb[:], in_=o16_d.ap())
            rep16_sb = big.tile([16, 128], F32, tag="rep16")
            nc.sync.dma_start(out=rep16_sb[:], in_=rep16_d.ap())
            o1_sb = big.tile([1, 128], F32, tag="o1")
            nc.sync.dma_start(out=o1_sb[:], in_=o1_d.ap())

            # routing state b lives in the a-pieces layout: [16, T, c],
            # partition m + tile column T <-> local route r = 16*T + m
            b_sb = big.tile([16, NT, C], F32, tag="b_state")
            nc.vector.memset(b_sb[:], 0.0)

            M_sb = big.tile([128, NT, CO], MMDT, tag="M")
            Gbig = big.tile([128, NT, CO], MMDT, tag="G")
            Pp = big.tile([128, NT, CO], MMDT, tag="Pp")
            P2 = big.tile([128, NT, C], F32, tag="P2")
            cE = big.tile([128, NT, C], MMDT, tag="cE")

            for it in range(NITER):
                # ---- unnormalized softmax from b state (iters >= 1) ----
                if it > 0:
                    exp_sb = sml.tile([16, NT, C], F32, tag="exp")
                    nc.scalar.activation(out=exp_sb[:], in_=b_sb[:],
                                         func=mybir.ActivationFunctionType.Exp)
                    sig_ps = ps_m.tile([128, NT * C], F32, tag="ps_small", name="sig_ps")[0:1]
                    nc.tensor.matmul(sig_ps[:], lhsT=o16_sb[:],
                                     rhs=exp_sb[:].rearrange("p t c -> p (t c)"),
                                     start=True, stop=True)
                    sig_sb = sml.tile([1, 10], F32, tag="sigloc")
                    nc.vector.tensor_reduce(
                        out=sig_sb[:],
                        in_=sig_ps[:].rearrange("p (t c) -> p c t", c=10),
                        axis=mybir.AxisListType.X, op=mybir.AluOpType.add)
                    # expand rows: cE[16*i + j, t, c] = exp[j, t, c] via one
                    # constant replication matmul on the PE
                    cE_ps = ps_m.tile([128, NT * C], F32, tag="ps_mid", name="cE_ps")
                    nc.tensor.matmul(cE_ps[:], lhsT=rep16_sb[:],
                                     rhs=exp_sb[:].rearrange("p t c -> p (t c)"),
                                     start=True, stop=True)
                    nc.scalar.copy(out=cE[:].rearrange("p t c -> p (t c)"),
                                   in_=cE_ps[:])
                    # M = W * cE (broadcast over o), chunked for PE overlap
                    for ch in range(3):
                        tsl = slice(6 * ch, 6 * ch + 6)
                        nc.vector.tensor_tensor(
                            out=M_sb[:, tsl, :].rearrange("p t (c o) -> p t c o", o=O),
                            in0=W_sb[:, tsl, :].rearrange("p t (c o) -> p t c o", o=O),
                            in1=cE[:, tsl, :, None].to_broadcast((128, 6, C, O)),
                            op=mybir.AluOpType.mult)
                else:
                    sig_sb = sml.tile([1, 10], F32, tag="sigloc")
                    nc.vector.memset(sig_sb[:], float(RL))

                # ---- s partial matmuls ----
                rhs_src = W_sb if it == 0 else M_sb
                psum_s = [ps_s.tile([128, CO], F32, tag="psum_s", name=f"psum_s{_h}") for _h in range(2)]
                for h in range(2):
                    for t in range(NT):
                        mm = nc.tensor.matmul(
                            psum_s[h][:],
                            lhsT=xT_sb[:, t, 128 * h:128 * h + 128],
                            rhs=rhs_src[:, t, :],
                            start=(t == 0), stop=(t == NT - 1))
                        if it == 0 and h == 0 and t == 0:
                            xb_dma = nc.sync.dma_start(
                                out=xb_sb[:],
                                in_=xb_d.ap().rearrange("(h p) k -> p h k", p=128))
                            tile.add_dep_helper(
                                xb_dma.ins, mm.ins,
                                reason="defer xb load until W/xT loads are done")

                sp_sb = sml.tile([128, 2, CO], F32, tag="sp_sb")
                for h in range(2):
                    nc.scalar.copy(out=sp_sb[:, h, :], in_=psum_s[h][:])

                if it == NITER - 1:
                    nc.sync.dma_start(
                        out=sout_d.ap().rearrange("(h p) f -> p h f", p=128),
                        in_=sp_sb[:])
                    nc.sync.dma_start(out=sigout_d.ap()[:, 0:10], in_=sig_sb[:])
                    break

                # ---- AllReduce payload: s parts + sigma ----
                ar_in = dram.tile([PAY], F32, tag="ar_in")
                ar_out = dram.tile([PAY], F32, tag="ar_out", addr_space="Shared")
                nc.sync.dma_start(
                    out=ar_in[0:B * CO].rearrange("(h p f) -> p h f", p=128, f=CO),
                    in_=sp_sb[:])
                nc.sync.dma_start(
                    out=ar_in[SIG_OFF:SIG_OFF + 10].rearrange("(p f) -> p f", p=1),
                    in_=sig_sb[:])
                nc.gpsimd.collective_compute(
                    "AllReduce", mybir.AluOpType.add,
                    replica_groups=[list(range(NCORE))],
                    ins=[ar_in.opt()], outs=[ar_out.opt()])

                s_sb = sml.tile([128, 2, CO], F32, tag="s_sb")
                nc.sync.dma_start(
                    out=s_sb[:],
                    in_=ar_out[0:B * CO].rearrange("(h p f) -> p h f", p=128, f=CO))
                sig_all = sml.tile([1, 10], F32, tag="sig_all")
                nc.sync.dma_start(
                    out=sig_all[:],
                    in_=ar_out[SIG_OFF:SIG_OFF + 10].rearrange("(p f) -> p f", p=1))

                # ---- squash: v = s_u * sqrt(sq)/(1+sq) / sigma,
                #      sq = (sum_o s_u^2)/sigma^2 ----
                sigB_ps = ps_m.tile([128, NT * C], F32, tag="ps_mid", name="sigB_ps")[:, 0:10]
                nc.tensor.matmul(sigB_ps[:], lhsT=o1_sb[:], rhs=sig_all[:],
                                 start=True, stop=True)
                sigB = sml.tile([128, 10], F32, tag="sigB")
                nc.scalar.copy(out=sigB[:], in_=sigB_ps[:])
                sig2 = sml.tile([128, 10], F32, tag="sig2")
                nc.vector.tensor_tensor(out=sig2[:], in0=sigB[:], in1=sigB[:],
                                        op=mybir.AluOpType.mult)
                ssq = sml.tile([128, 2, CO], F32, tag="ssq")
                nc.vector.tensor_tensor(out=ssq[:], in0=s_sb[:], in1=s_sb[:],
                                        op=mybir.AluOpType.mult)
                t2 = sml.tile([128, 2, C], F32, tag="t2")
                nc.vector.tensor_reduce(
                    out=t2[:], in_=ssq[:].rearrange("p h (c o) -> p h c o", o=O),
                    axis=mybir.AxisListType.X, op=mybir.AluOpType.add)
                # g = sqrt(t2) / (sigma^2 + t2); then v = s_u * g
                rt = sml.tile([128, 2, C], F32, tag="rt")
                nc.scalar.activation(out=rt[:], in_=t2[:],
                                     func=mybir.ActivationFunctionType.Sqrt)
                dn = sml.tile([128, 2, C], F32, tag="dn")
                nc.vector.tensor_tensor(
                    out=dn[:], in0=t2[:],
                    in1=sig2[:, None, :].to_broadcast((128, 2, C)),
                    op=mybir.AluOpType.add)
                nc.vector.reciprocal(out=dn[:], in_=dn[:])
                g_f = sml.tile([128, 2, C], F32, tag="g_f")
                nc.vector.tensor_tensor(out=g_f[:], in0=rt[:], in1=dn[:],
                                        op=mybir.AluOpType.mult)
                v_sb = sml.tile([128, 2, CO], MMDT, tag="v_sb")
                nc.vector.tensor_tensor(
                    out=v_sb[:].rearrange("p h (c o) -> p h c o", o=O),
                    in0=s_sb[:].rearrange("p h (c o) -> p h c o", o=O),
                    in1=g_f[:, :, :, None].to_broadcast((128, 2, C, O)),
                    op=mybir.AluOpType.mult)

                # ---- G = x^T v, then a = (1/B) sum_{i,o} W*G ----
                a_stage = sml.tile([16, NT, C], F32, tag="a_stage")
                for ch in range(3):
                    tsl = slice(6 * ch, 6 * ch + 6)
                    for t in range(6 * ch, 6 * ch + 6):
                        G_ps = ps_g.tile([128, CO], F32, tag="G_ps")
                        for h in range(2):
                            nc.tensor.matmul(
                                G_ps[:],
                                lhsT=xb_sb[:, h, 128 * t:128 * t + 128],
                                rhs=v_sb[:, h, :],
                                start=(h == 0), stop=(h == 1))
                        nc.scalar.copy(out=Gbig[:, t, :], in_=G_ps[:])
                    nc.vector.tensor_tensor(out=Pp[:, tsl, :], in0=Gbig[:, tsl, :],
                                            in1=W_sb[:, tsl, :],
                                            op=mybir.AluOpType.mult)
                    nc.vector.tensor_reduce(
                        out=P2[:, tsl, :],
                        in_=Pp[:, tsl, :].rearrange("p t (c o) -> p t c o", o=O),
                        axis=mybir.AxisListType.X, op=mybir.AluOpType.add)
                    ap_ps = ps_m.tile([128, NT * C], F32, tag="ps_small", name="ap_ps")[0:16, 0:60]
                    nc.tensor.matmul(
                        ap_ps[:],
                        lhsT=oblk_sb[:],
                        rhs=P2[:, tsl, :].rearrange("p t c -> p (t c)"),
                        start=True, stop=True)
                    nc.scalar.copy(
                        out=a_stage[:, tsl, :],
                        in_=ap_ps[:].rearrange("m (t c) -> m t c", c=10))
                nc.vector.tensor_tensor(out=b_sb[:], in0=b_sb[:], in1=a_stage[:],
                                        op=mybir.AluOpType.add)

    nc.compile()
    return nc


def _get_nc():
    if _CACHE["nc"] is None:
        _CACHE["nc"] = _build()
    return _CACHE["nc"]


def _row_major(a4):
    """[T, j(16), i(8), ...] -> [T, i, j, ...] flattened rows: row = i*16+j."""
    return np.ascontiguousarray(a4.transpose(0, 2, 1, *range(3, a4.ndim)))


def kernel(x, W):
    global LAST_RESULT
    x = np.ascontiguousarray(np.asarray(x), dtype=np.float32)
    W = np.ascontiguousarray(np.asarray(W), dtype=np.float32)
    assert x.shape == (B, R, I) and W.shape == (R, C, O, I)

    nc = _get_nc()

    # [(r i), (c o)] with i-major row order within each 128-row tile
    Wp = W.transpose(0, 3, 1, 2).reshape(R // 16, 16, I, CO)
    Wp = _row_major(Wp).reshape(R * I, CO)
    oblk = np.zeros((128, 16), np.float32)
    for k in range(128):
        oblk[k, k % 16] = 1.0 / B      # row k = i*16+j -> j = k % 16
    o16 = np.ones((16, 1), np.float32)
    rep16 = np.zeros((16, 128), np.float32)
    for p in range(128):
        rep16[p % 16, p] = 1.0
    o1 = np.ones((1, 128), np.float32)

    import ml_dtypes
    mdt = ml_dtypes.bfloat16 if USE_BF16 else np.float32
    in_maps = []
    for cid in range(NCORE):
        xs = x[:, cid * RL:(cid + 1) * RL, :]                     # [B, 288, 8]
        xT = xs.transpose(1, 2, 0).reshape(NT, 16, I, B)          # [T, j, i, B]
        xT = _row_major(xT).reshape(KI, B)
        xbv = xs.reshape(B, NT, 16, I).transpose(0, 1, 3, 2)      # [B, T, i, j]
        xbv = np.ascontiguousarray(xbv).reshape(B, KI)
        in_maps.append({
            "xT": np.ascontiguousarray(xT, dtype=mdt),
            "xb": np.ascontiguousarray(xbv, dtype=mdt),
            "Wl": np.ascontiguousarray(Wp[cid * KI:(cid + 1) * KI], dtype=mdt),
            "ones_blk": oblk,
            "ones16": o16,
            "rep16": rep16,
            "ones1": o1,
        })

    res = bass_utils.run_bass_kernel_spmd(
        nc, in_maps, core_ids=list(range(NCORE)),
        trace=bool(os.environ.get("DIGITCAPS_TRACE")))
    LAST_RESULT = res

    s2 = np.zeros((B, CO), np.float64)
    sig = np.zeros((10,), np.float64)
    for cid in range(NCORE):
        s2 += res.results[cid]["sout"]
        sig += res.results[cid]["sigout"][0, :10]
    s3 = (s2.reshape(B, C, O) / sig[None, :, None]).astype(np.float32)
    sq = (s3 * s3).sum(axis=2, keepdims=True)
    v = s3 * (np.sqrt(sq) / (1.0 + sq))
    return v[..., None].astype(np.float32)


# revision 15
# speedup vs baseline: 1.4133x; 1.0238x over previous
"""DigitCaps dynamic-routing kernel for 8 Trainium2 NeuronCores.

Decomposition (validated against the jax reference in a numpy mock):
  u_hat[b,r,c,o] = sum_i W[r,c,o,i] x[b,r,i]   (never materialized)
  s[b,c,o]   = sum_{r,i} x[b,r,i] * cexp[r,c] * W[r,c,o,i] / sigma[c]
  a[r,c]     = (1/B) sum_{i,o} W[r,c,o,i] * G[r,i,c,o],
               G[(r,i),(c,o)] = sum_b x[b,r,i] v[b,c,o]
  softmax over r is computed unnormalized (cexp = exp(b), safe: |b| < 1);
  the divide by sigma[c] = sum_r exp(b[r,c]) folds into the squash.

Sharding: 8-way over routes R (288 each). Each core holds its W shard and
the full batch of x for its routes, so `a` needs no collective. The only
cross-core exchange is the per-iteration AllReduce of the partial
s[b,(c,o)] (plus the 10 local sigma values in the same payload). The final
iteration's s-partials are returned per core; the host sums them and
applies the (tiny) squash while unsharding.

SBUF row space per core: 18 tiles of 128 rows; tile t row (i*16+j) holds
(r = 288*core + 16*t + j, i) — i-major so the exp(b) -> row expansion DMA
writes contiguous partition blocks.
"""
import os
import sys

import numpy as np

_REPO = "/opt/trn_rl_repo"
if _REPO not in sys.path:
    sys.path.insert(0, _REPO)

import concourse.bass as bass  # noqa: E402,F401
import concourse.tile as tile  # noqa: E402
from concourse import bacc, mybir  # noqa: E402
from concourse import bass_utils  # noqa: E402

B, R, C, O, I = 256, 2304, 10, 16, 8
NCORE = 8
RL = R // NCORE          # 288 routes per core
KI = RL * I              # 2304 (r,i) rows per core
NT = KI // 128           # 18 tiles of 128 (r,i) rows
CO = C * O               # 160
NITER = 3
F32 = mybir.dt.float32
BF16 = mybir.dt.bfloat16
USE_BF16 = not os.environ.get("DIGITCAPS_FP32")
MMDT = BF16 if USE_BF16 else F32
SIG_OFF = B * CO         # float offset of the sigma slot in the AR payload
PAY = SIG_OFF + 16       # AR payload floats (s parts + sigma + pad)

_CACHE = {"nc": None}
LAST_RESULT = None


def _build():
    nc = bacc.Bacc("TRN2", target_bir_lowering=False, debug=False, num_devices=NCORE)

    xT_d = nc.dram_tensor("xT", [KI, B], MMDT, kind="ExternalInput")
    xb_d = nc.dram_tensor("xb", [B, KI], MMDT, kind="ExternalInput")
    W_d = nc.dram_tensor("Wl", [KI, CO], MMDT, kind="ExternalInput")
    oblk_d = nc.dram_tensor("ones_blk", [128, 16], F32, kind="ExternalInput")
    o16_d = nc.dram_tensor("ones16", [16, 1], F32, kind="ExternalInput")
    rep16_d = nc.dram_tensor("rep16", [16, 128], F32, kind="ExternalInput")
    o1_d = nc.dram_tensor("ones1", [1, 128], MMDT, kind="ExternalInput")
    sout_d = nc.dram_tensor("sout", [B, CO], F32, kind="ExternalOutput")
    sigout_d = nc.dram_tensor("sigout", [1, 16], F32, kind="ExternalOutput")

    with tile.TileContext(nc) as tc:
        with tc.tile_pool(name="big", bufs=1) as big, \
             tc.tile_pool(name="sml", bufs=2) as sml, \
             tc.tile_pool(name="ps_s", bufs=2, space="PSUM") as ps_s, \
             tc.tile_pool(name="ps_g", bufs=4, space="PSUM") as ps_g, \
             tc.tile_pool(name="ps_m", bufs=1, space="PSUM") as ps_m, \
             tc.tile_pool(name="dram", bufs=2, space="DRAM") as dram:

            # ---- preload (chunked so the first matmuls start early) ----
            W_sb = big.tile([128, NT, CO], MMDT, tag="W")
            xT_sb = big.tile([128, NT, B], MMDT, tag="xT")
            for ch in range(3):
                tsl = slice(6 * ch, 6 * ch + 6)
                nc.sync.dma_start(
                    out=W_sb[:, tsl, :],
                    in_=W_d.ap().rearrange("(t p) f -> p t f", p=128)[:, tsl, :])
                nc.sync.dma_start(
                    out=xT_sb[:, tsl, :],
                    in_=xT_d.ap().rearrange("(t p) b -> p t b", p=128)[:, tsl, :])
            xb_sb = big.tile([128, 2, KI], MMDT, tag="xb")
            xb_dma = None   # deferred until the first s0 matmul has its data
            oblk_sb = big.tile([128, 16], F32, tag="oblk")
            nc.sync.dma_start(out=oblk_sb[:], in_=oblk_d.ap())
            o16_sb = big.tile([16, 1], F32, tag="o16")
            nc.sync.dma_start(out=o16_sb[:], in_=o16_d.ap())
            rep16_sb = big.tile([16, 128], F32, tag="rep16")
            nc.sync.dma_start(out=rep16_sb[:], in_=rep16_d.ap())
            o1_sb = big.tile([1, 128], MMDT, tag="o1")
            nc.sync.dma_start(out=o1_sb[:], in_=o1_d.ap())

            # routing state b lives in the a-pieces layout: [16, T, c],
            # partition m + tile column T <-> local route r = 16*T + m
            b_sb = big.tile([16, NT, C], F32, tag="b_state")
            nc.vector.memset(b_sb[:], 0.0)

            M_sb = big.tile([128, NT, CO], MMDT, tag="M")
            Gbig = big.tile([128, NT, CO], MMDT, tag="G")
            Pp = big.tile([128, NT, CO], MMDT, tag="Pp")
            P2 = big.tile([128, NT, C], F32, tag="P2")
            cE = big.tile([128, NT, C], MMDT, tag="cE")

            for it in range(NITER):
                # ---- unnormalized softmax from b state (iters >= 1) ----
                if it > 0:
                    exp_sb = sml.tile([16, NT, C], F32, tag="exp")
                    nc.scalar.activation(out=exp_sb[:], in_=b_sb[:],
                                         func=mybir.ActivationFunctionType.Exp)
                    sig_ps = ps_m.tile([128, NT * C], F32, tag="ps_small", name="sig_ps")[0:1]
                    nc.tensor.matmul(sig_ps[:], lhsT=o16_sb[:],
                                     rhs=exp_sb[:].rearrange("p t c -> p (t c)"),
                                     start=True, stop=True)
                    sig_sb = sml.tile([1, 10], F32, tag="sigloc")
                    nc.vector.tensor_reduce(
                        out=sig_sb[:],
                        in_=sig_ps[:].rearrange("p (t c) -> p c t", c=10),
                        axis=mybir.AxisListType.X, op=mybir.AluOpType.add)
                    # expand rows: cE[16*i + j, t, c] = exp[j, t, c] via one
                    # constant replication matmul on the PE
                    cE_ps = ps_m.tile([128, NT * C], F32, tag="ps_mid", name="cE_ps")
                    nc.tensor.matmul(cE_ps[:], lhsT=rep16_sb[:],
                                     rhs=exp_sb[:].rearrange("p t c -> p (t c)"),
                                     start=True, stop=True)
                    nc.scalar.copy(out=cE[:].rearrange("p t c -> p (t c)"),
                                   in_=cE_ps[:])
                    # M = W * cE (broadcast over o), chunked for PE overlap
                    for ch in range(3):
                        tsl = slice(6 * ch, 6 * ch + 6)
                        nc.vector.tensor_tensor(
                            out=M_sb[:, tsl, :].rearrange("p t (c o) -> p t c o", o=O),
                            in0=W_sb[:, tsl, :].rearrange("p t (c o) -> p t c o", o=O),
                            in1=cE[:, tsl, :, None].to_broadcast((128, 6, C, O)),
                            op=mybir.AluOpType.mult)
                else:
                    sig_sb = sml.tile([1, 10], F32, tag="sigloc")
                    nc.vector.memset(sig_sb[:], float(RL))

                # ---- s partial matmuls ----
                rhs_src = W_sb if it == 0 else M_sb
                psum_s = [ps_s.tile([128, CO], F32, tag="psum_s", name=f"psum_s{_h}") for _h in range(2)]
                for h in range(2):
                    for t in range(NT):
                        mm = nc.tensor.matmul(
                            psum_s[h][:],
                            lhsT=xT_sb[:, t, 128 * h:128 * h + 128],
                            rhs=rhs_src[:, t, :],
                            start=(t == 0), stop=(t == NT - 1))
                        if it == 0 and h == 0 and t == 0:
                            xb_dma = nc.sync.dma_start(
                                out=xb_sb[:],
                                in_=xb_d.ap().rearrange("(h p) k -> p h k", p=128))
                            tile.add_dep_helper(
                                xb_dma.ins, mm.ins,
                                reason="defer xb load until W/xT loads are done")

                if it == NITER - 1:
                    sp_sb = sml.tile([128, 2, CO], F32, tag="sp_sb")
                    for h in range(2):
                        nc.scalar.copy(out=sp_sb[:, h, :], in_=psum_s[h][:])
                    nc.sync.dma_start(
                        out=sout_d.ap().rearrange("(h p) f -> p h f", p=128),
                        in_=sp_sb[:])
                    nc.sync.dma_start(out=sigout_d.ap()[:, 0:10], in_=sig_sb[:])
                    break

                # ---- AllReduce payload (bf16): s parts + sigma ----
                sp_bf = sml.tile([128, 2, CO], MMDT, tag="sp_bf")
                for h in range(2):
                    nc.scalar.copy(out=sp_bf[:, h, :], in_=psum_s[h][:])
                sig_bf = sml.tile([1, 10], MMDT, tag="sig_bf")
                nc.vector.tensor_copy(out=sig_bf[:], in_=sig_sb[:])
                ar_in = dram.tile([PAY], MMDT, tag="ar_in")
                ar_out = dram.tile([PAY], MMDT, tag="ar_out", addr_space="Shared")
                nc.sync.dma_start(
                    out=ar_in[0:B * CO].rearrange("(h p f) -> p h f", p=128, f=CO),
                    in_=sp_bf[:])
                nc.sync.dma_start(
                    out=ar_in[SIG_OFF:SIG_OFF + 10].rearrange("(p f) -> p f", p=1),
                    in_=sig_bf[:])
                nc.gpsimd.collective_compute(
                    "AllReduce", mybir.AluOpType.add,
                    replica_groups=[list(range(NCORE))],
                    ins=[ar_in.opt()], outs=[ar_out.opt()])

                s_sb = sml.tile([128, 2, CO], MMDT, tag="s_sb")
                nc.sync.dma_start(
                    out=s_sb[:],
                    in_=ar_out[0:B * CO].rearrange("(h p f) -> p h f", p=128, f=CO))
                sig_all = sml.tile([1, 10], MMDT, tag="sig_all")
                nc.sync.dma_start(
                    out=sig_all[:],
                    in_=ar_out[SIG_OFF:SIG_OFF + 10].rearrange("(p f) -> p f", p=1))

                # ---- squash: v = s_u * sqrt(sq)/(1+sq) / sigma,
                #      sq = (sum_o s_u^2)/sigma^2 ----
                sigB_ps = ps_m.tile([128, NT * C], F32, tag="ps_mid", name="sigB_ps")[:, 0:10]
                nc.tensor.matmul(sigB_ps[:], lhsT=o1_sb[:], rhs=sig_all[:],
                                 start=True, stop=True)
                sigB = sml.tile([128, 10], F32, tag="sigB")
                nc.scalar.copy(out=sigB[:], in_=sigB_ps[:])
                sig2 = sml.tile([128, 10], F32, tag="sig2")
                nc.vector.tensor_tensor(out=sig2[:], in0=sigB[:], in1=sigB[:],
                                        op=mybir.AluOpType.mult)
                ssq = sml.tile([128, 2, CO], MMDT, tag="ssq")
                nc.vector.tensor_tensor(out=ssq[:], in0=s_sb[:], in1=s_sb[:],
                                        op=mybir.AluOpType.mult)
                t2 = sml.tile([128, 2, C], F32, tag="t2")
                nc.vector.tensor_reduce(
                    out=t2[:], in_=ssq[:].rearrange("p h (c o) -> p h c o", o=O),
                    axis=mybir.AxisListType.X, op=mybir.AluOpType.add)
                # g = sqrt(t2) / (sigma^2 + t2); then v = s_u * g
                rt = sml.tile([128, 2, C], F32, tag="rt")
                nc.scalar.activation(out=rt[:], in_=t2[:],
                                     func=mybir.ActivationFunctionType.Sqrt)
                dn = sml.tile([128, 2, C], F32, tag="dn")
                nc.vector.tensor_tensor(
                    out=dn[:], in0=t2[:],
                    in1=sig2[:, None, :].to_broadcast((128, 2, C)),
                    op=mybir.AluOpType.add)
                nc.vector.reciprocal(out=dn[:], in_=dn[:])
                g_f = sml.tile([128, 2, C], F32, tag="g_f")
                nc.vector.tensor_tensor(out=g_f[:], in0=rt[:], in1=dn[:],
                                        op=mybir.AluOpType.mult)
                v_sb = sml.tile([128, 2, CO], MMDT, tag="v_sb")
                nc.vector.tensor_tensor(
                    out=v_sb[:].rearrange("p h (c o) -> p h c o", o=O),
                    in0=s_sb[:].rearrange("p h (c o) -> p h c o", o=O),
                    in1=g_f[:, :, :, None].to_broadcast((128, 2, C, O)),
                    op=mybir.AluOpType.mult)

                # ---- G = x^T v, then a = (1/B) sum_{i,o} W*G ----
                a_stage = sml.tile([16, NT, C], F32, tag="a_stage")
                for ch in range(3):
                    tsl = slice(6 * ch, 6 * ch + 6)
                    for t in range(6 * ch, 6 * ch + 6):
                        G_ps = ps_g.tile([128, CO], F32, tag="G_ps")
                        for h in range(2):
                            nc.tensor.matmul(
                                G_ps[:],
                                lhsT=xb_sb[:, h, 128 * t:128 * t + 128],
                                rhs=v_sb[:, h, :],
                                start=(h == 0), stop=(h == 1))
                        if t % 2 == 0:
                            nc.scalar.copy(out=Gbig[:, t, :], in_=G_ps[:])
                        else:
                            nc.vector.tensor_copy(out=Gbig[:, t, :], in_=G_ps[:])
                    nc.vector.tensor_tensor(out=Pp[:, tsl, :], in0=Gbig[:, tsl, :],
                                            in1=W_sb[:, tsl, :],
                                            op=mybir.AluOpType.mult)
                    nc.vector.tensor_reduce(
                        out=P2[:, tsl, :],
                        in_=Pp[:, tsl, :].rearrange("p t (c o) -> p t c o", o=O),
                        axis=mybir.AxisListType.X, op=mybir.AluOpType.add)
                    ap_ps = ps_m.tile([128, NT * C], F32, tag="ps_small", name="ap_ps")[0:16, 0:60]
                    nc.tensor.matmul(
                        ap_ps[:],
                        lhsT=oblk_sb[:],
                        rhs=P2[:, tsl, :].rearrange("p t c -> p (t c)"),
                        start=True, stop=True)
                    nc.scalar.copy(
                        out=a_stage[:, tsl, :],
                        in_=ap_ps[:].rearrange("m (t c) -> m t c", c=10))
                nc.vector.tensor_tensor(out=b_sb[:], in0=b_sb[:], in1=a_stage[:],
                                        op=mybir.AluOpType.add)

    nc.compile()
    return nc


def _get_nc():
    if _CACHE["nc"] is None:
        _CACHE["nc"] = _build()
    return _CACHE["nc"]


def _row_major(a4):
    """[T, j(16), i(8), ...] -> [T, i, j, ...] flattened rows: row = i*16+j."""
    return np.ascontiguousarray(a4.transpose(0, 2, 1, *range(3, a4.ndim)))


def kernel(x, W):
    global LAST_RESULT
    x = np.ascontiguousarray(np.asarray(x), dtype=np.float32)
    W = np.ascontiguousarray(np.asarray(W), dtype=np.float32)
    assert x.shape == (B, R, I) and W.shape == (R, C, O, I)

    nc = _get_nc()

    # [(r i), (c o)] with i-major row order within each 128-row tile
    Wp = W.transpose(0, 3, 1, 2).reshape(R // 16, 16, I, CO)
    Wp = _row_major(Wp).reshape(R * I, CO)
    oblk = np.zeros((128, 16), np.float32)
    for k in range(128):
        oblk[k, k % 16] = 1.0 / B      # row k = i*16+j -> j = k % 16
    o16 = np.ones((16, 1), np.float32)
    rep16 = np.zeros((16, 128), np.float32)
    for p in range(128):
        rep16[p % 16, p] = 1.0
    o1 = np.ones((1, 128), np.float32)

    import ml_dtypes
    mdt = ml_dtypes.bfloat16 if USE_BF16 else np.float32
    in_maps = []
    for cid in range(NCORE):
        xs = x[:, cid * RL:(cid + 1) * RL, :]                     # [B, 288, 8]
        xT = xs.transpose(1, 2, 0).reshape(NT, 16, I, B)          # [T, j, i, B]
        xT = _row_major(xT).reshape(KI, B)
        xbv = xs.reshape(B, NT, 16, I).transpose(0, 1, 3, 2)      # [B, T, i, j]
        xbv = np.ascontiguousarray(xbv).reshape(B, KI)
        in_maps.append({
            "xT": np.ascontiguousarray(xT, dtype=mdt),
            "xb": np.ascontiguousarray(xbv, dtype=mdt),
            "Wl": np.ascontiguousarray(Wp[cid * KI:(cid + 1) * KI], dtype=mdt),
            "ones_blk": oblk,
            "ones16": o16,
            "rep16": rep16,
            "ones1": np.ascontiguousarray(o1, dtype=mdt),
        })

    res = bass_utils.run_bass_kernel_spmd(
        nc, in_maps, core_ids=list(range(NCORE)),
        trace=bool(os.environ.get("DIGITCAPS_TRACE")))
    LAST_RESULT = res

    s2 = np.zeros((B, CO), np.float64)
    sig = np.zeros((10,), np.float64)
    for cid in range(NCORE):
        s2 += res.results[cid]["sout"]
        sig += res.results[cid]["sigout"][0, :10]
    s3 = (s2.reshape(B, C, O) / sig[None, :, None]).astype(np.float32)
    sq = (s3 * s3).sum(axis=2, keepdims=True)
    v = s3 * (np.sqrt(sq) / (1.0 + sq))
    return v[..., None].astype(np.float32)


# revision 16
# speedup vs baseline: 1.4973x; 1.0594x over previous
"""DigitCaps dynamic-routing kernel for 8 Trainium2 NeuronCores.

Decomposition (validated against the jax reference in a numpy mock):
  u_hat[b,r,c,o] = sum_i W[r,c,o,i] x[b,r,i]   (never materialized)
  s[b,c,o]   = sum_{r,i} x[b,r,i] * cexp[r,c] * W[r,c,o,i] / sigma[c]
  a[r,c]     = (1/B) sum_{i,o} W[r,c,o,i] * G[r,i,c,o],
               G[(r,i),(c,o)] = sum_b x[b,r,i] v[b,c,o]
  softmax over r is computed unnormalized (cexp = exp(b), safe: |b| < 1);
  the divide by sigma[c] = sum_r exp(b[r,c]) folds into the squash.

Sharding: 8-way over routes R (288 each). Each core holds its W shard and
the full batch of x for its routes, so `a` needs no collective. The only
cross-core exchange is the per-iteration AllReduce of the partial
s[b,(c,o)] (plus the 10 local sigma values in the same payload). The final
iteration's s-partials are returned per core; the host sums them and
applies the (tiny) squash while unsharding.

SBUF row space per core: 18 tiles of 128 rows; tile t row (i*16+j) holds
(r = 288*core + 16*t + j, i) — i-major so the exp(b) -> row expansion DMA
writes contiguous partition blocks.
"""
import os
import sys

import numpy as np

_REPO = "/opt/trn_rl_repo"
if _REPO not in sys.path:
    sys.path.insert(0, _REPO)

import concourse.bass as bass  # noqa: E402,F401
import concourse.tile as tile  # noqa: E402
from concourse import bacc, mybir  # noqa: E402
from concourse import bass_utils  # noqa: E402

B, R, C, O, I = 256, 2304, 10, 16, 8
NCORE = 8
RL = R // NCORE          # 288 routes per core
KI = RL * I              # 2304 (r,i) rows per core
NT = KI // 128           # 18 tiles of 128 (r,i) rows
CO = C * O               # 160
NITER = 3
F32 = mybir.dt.float32
BF16 = mybir.dt.bfloat16
USE_BF16 = not os.environ.get("DIGITCAPS_FP32")
MMDT = BF16 if USE_BF16 else F32
SIG_OFF = B * CO         # float offset of the sigma slot in the AR payload
PAY = SIG_OFF + 16       # AR payload floats (s parts + sigma + pad)

_CACHE = {"nc": None}
LAST_RESULT = None


def _build():
    nc = bacc.Bacc("TRN2", target_bir_lowering=False, debug=False, num_devices=NCORE)

    xT_d = nc.dram_tensor("xT", [KI, B], MMDT, kind="ExternalInput")
    xb_d = nc.dram_tensor("xb", [B, KI], MMDT, kind="ExternalInput")
    W_d = nc.dram_tensor("Wl", [KI, CO], MMDT, kind="ExternalInput")
    oblk_d = nc.dram_tensor("ones_blk", [128, 16], F32, kind="ExternalInput")
    o16_d = nc.dram_tensor("ones16", [16, 1], F32, kind="ExternalInput")
    rep16_d = nc.dram_tensor("rep16", [16, 128], F32, kind="ExternalInput")
    o1_d = nc.dram_tensor("ones1", [1, 128], MMDT, kind="ExternalInput")
    sout_d = nc.dram_tensor("sout", [B, CO], F32, kind="ExternalOutput")
    sigout_d = nc.dram_tensor("sigout", [1, 16], F32, kind="ExternalOutput")

    with tile.TileContext(nc) as tc:
        with tc.tile_pool(name="big", bufs=1) as big, \
             tc.tile_pool(name="sml", bufs=2) as sml, \
             tc.tile_pool(name="ps_s", bufs=2, space="PSUM") as ps_s, \
             tc.tile_pool(name="ps_g", bufs=4, space="PSUM") as ps_g, \
             tc.tile_pool(name="ps_m", bufs=1, space="PSUM") as ps_m, \
             tc.tile_pool(name="dram", bufs=2, space="DRAM") as dram:

            # ---- preload (chunked so the first matmuls start early) ----
            W_sb = big.tile([128, NT, CO], MMDT, tag="W")
            xT_sb = big.tile([128, NT, B], MMDT, tag="xT")
            for ch in range(3):
                tsl = slice(6 * ch, 6 * ch + 6)
                nc.sync.dma_start(
                    out=W_sb[:, tsl, :],
                    in_=W_d.ap().rearrange("(t p) f -> p t f", p=128)[:, tsl, :])
                nc.sync.dma_start(
                    out=xT_sb[:, tsl, :],
                    in_=xT_d.ap().rearrange("(t p) b -> p t b", p=128)[:, tsl, :])
            xb_sb = big.tile([128, 2, KI], MMDT, tag="xb")
            xb_dma = None   # deferred until the first s0 matmul has its data
            oblk_sb = big.tile([128, 16], F32, tag="oblk")
            nc.sync.dma_start(out=oblk_sb[:], in_=oblk_d.ap())
            o16_sb = big.tile([16, 1], F32, tag="o16")
            nc.sync.dma_start(out=o16_sb[:], in_=o16_d.ap())
            rep16_sb = big.tile([16, 128], F32, tag="rep16")
            nc.sync.dma_start(out=rep16_sb[:], in_=rep16_d.ap())
            o1_sb = big.tile([1, 128], MMDT, tag="o1")
            nc.sync.dma_start(out=o1_sb[:], in_=o1_d.ap())

            # routing state b lives in the a-pieces layout: [16, T, c],
            # partition m + tile column T <-> local route r = 16*T + m
            b_sb = big.tile([16, NT, C], F32, tag="b_state")
            nc.vector.memset(b_sb[:], 0.0)

            M_sb = big.tile([128, NT, CO], MMDT, tag="M")
            Gbig = big.tile([128, NT, CO], MMDT, tag="G")
            Pp = big.tile([128, NT, CO], MMDT, tag="Pp")
            P2 = big.tile([128, NT, C], F32, tag="P2")
            cE = big.tile([128, NT, C], MMDT, tag="cE")

            for it in range(NITER):
                # ---- unnormalized softmax from b state (iters >= 1) ----
                if it > 0:
                    exp_sb = sml.tile([16, NT, C], F32, tag="exp")
                    nc.scalar.activation(out=exp_sb[:], in_=b_sb[:],
                                         func=mybir.ActivationFunctionType.Exp)
                    sig_ps = ps_m.tile([128, NT * C], F32, tag="ps_small", name="sig_ps")[0:1]
                    nc.tensor.matmul(sig_ps[:], lhsT=o16_sb[:],
                                     rhs=exp_sb[:].rearrange("p t c -> p (t c)"),
                                     start=True, stop=True)
                    sig_sb = sml.tile([1, 10], F32, tag="sigloc")
                    nc.vector.tensor_reduce(
                        out=sig_sb[:],
                        in_=sig_ps[:].rearrange("p (t c) -> p c t", c=10),
                        axis=mybir.AxisListType.X, op=mybir.AluOpType.add)
                    # expand rows: cE[16*i + j, t, c] = exp[j, t, c] via one
                    # constant replication matmul on the PE
                    cE_ps = ps_m.tile([128, NT * C], F32, tag="ps_mid", name="cE_ps")
                    nc.tensor.matmul(cE_ps[:], lhsT=rep16_sb[:],
                                     rhs=exp_sb[:].rearrange("p t c -> p (t c)"),
                                     start=True, stop=True)
                    nc.scalar.copy(out=cE[:].rearrange("p t c -> p (t c)"),
                                   in_=cE_ps[:])
                    # M = W * cE (broadcast over o), chunked for PE overlap
                    for ch in range(3):
                        tsl = slice(6 * ch, 6 * ch + 6)
                        nc.vector.tensor_tensor(
                            out=M_sb[:, tsl, :].rearrange("p t (c o) -> p t c o", o=O),
                            in0=W_sb[:, tsl, :].rearrange("p t (c o) -> p t c o", o=O),
                            in1=cE[:, tsl, :, None].to_broadcast((128, 6, C, O)),
                            op=mybir.AluOpType.mult)
                else:
                    sig_sb = sml.tile([1, 10], F32, tag="sigloc")
                    nc.vector.memset(sig_sb[:], float(RL))

                # ---- s partial matmuls ----
                rhs_src = W_sb if it == 0 else M_sb
                psum_s = [ps_s.tile([128, CO], F32, tag="psum_s", name=f"psum_s{_h}") for _h in range(2)]
                for h in range(2):
                    for t in range(NT):
                        mm = nc.tensor.matmul(
                            psum_s[h][:],
                            lhsT=xT_sb[:, t, 128 * h:128 * h + 128],
                            rhs=rhs_src[:, t, :],
                            start=(t == 0), stop=(t == NT - 1))
                        if it == 0 and h == 0 and t == 0:
                            xb_dma = nc.sync.dma_start(
                                out=xb_sb[:],
                                in_=xb_d.ap().rearrange("(h p) k -> p h k", p=128))
                            tile.add_dep_helper(
                                xb_dma.ins, mm.ins,
                                reason="defer xb load until W/xT loads are done")

                if it == NITER - 1:
                    sp_sb = sml.tile([128, 2, CO], F32, tag="sp_sb")
                    for h in range(2):
                        nc.scalar.copy(out=sp_sb[:, h, :], in_=psum_s[h][:])
                    nc.sync.dma_start(
                        out=sout_d.ap().rearrange("(h p) f -> p h f", p=128),
                        in_=sp_sb[:])
                    nc.sync.dma_start(out=sigout_d.ap()[:, 0:10], in_=sig_sb[:])
                    break

                # ---- AllReduce payload (bf16): s parts + sigma ----
                sp_bf = sml.tile([128, 2, CO], MMDT, tag="sp_bf")
                for h in range(2):
                    nc.scalar.copy(out=sp_bf[:, h, :], in_=psum_s[h][:])
                sig_bf = sml.tile([1, 10], MMDT, tag="sig_bf")
                nc.vector.tensor_copy(out=sig_bf[:], in_=sig_sb[:])
                ar_in = dram.tile([PAY], MMDT, tag="ar_in")
                ar_out = dram.tile([PAY], MMDT, tag="ar_out", addr_space="Shared")
                nc.sync.dma_start(
                    out=ar_in[0:B * CO].rearrange("(h p f) -> p h f", p=128, f=CO),
                    in_=sp_bf[:])
                nc.sync.dma_start(
                    out=ar_in[SIG_OFF:SIG_OFF + 10].rearrange("(p f) -> p f", p=1),
                    in_=sig_bf[:])
                nc.gpsimd.collective_compute(
                    "AllReduce", mybir.AluOpType.add,
                    replica_groups=[list(range(NCORE))],
                    ins=[ar_in.opt()], outs=[ar_out.opt()])
                warm = sml.tile([1, 1], F32, tag="warm", name=f"warm_sq{it}")
                nc.scalar.activation(out=warm[:], in_=sig_sb[:, 0:1],
                                     func=mybir.ActivationFunctionType.Sqrt)

                s_sb = sml.tile([128, 2, CO], MMDT, tag="s_sb")
                nc.sync.dma_start(
                    out=s_sb[:],
                    in_=ar_out[0:B * CO].rearrange("(h p f) -> p h f", p=128, f=CO))
                sig_all = sml.tile([1, 10], MMDT, tag="sig_all")
                nc.sync.dma_start(
                    out=sig_all[:],
                    in_=ar_out[SIG_OFF:SIG_OFF + 10].rearrange("(p f) -> p f", p=1))

                # ---- squash: v = s_u * sqrt(sq)/(1+sq) / sigma,
                #      sq = (sum_o s_u^2)/sigma^2 ----
                sigB_ps = ps_m.tile([128, NT * C], F32, tag="ps_mid", name="sigB_ps")[:, 0:10]
                nc.tensor.matmul(sigB_ps[:], lhsT=o1_sb[:], rhs=sig_all[:],
                                 start=True, stop=True)
                sigB = sml.tile([128, 10], F32, tag="sigB")
                nc.scalar.copy(out=sigB[:], in_=sigB_ps[:])
                sig2 = sml.tile([128, 10], F32, tag="sig2")
                nc.vector.tensor_tensor(out=sig2[:], in0=sigB[:], in1=sigB[:],
                                        op=mybir.AluOpType.mult)
                ssq = sml.tile([128, 2, CO], MMDT, tag="ssq")
                nc.vector.tensor_tensor(out=ssq[:], in0=s_sb[:], in1=s_sb[:],
                                        op=mybir.AluOpType.mult)
                t2 = sml.tile([128, 2, C], F32, tag="t2")
                nc.vector.tensor_reduce(
                    out=t2[:], in_=ssq[:].rearrange("p h (c o) -> p h c o", o=O),
                    axis=mybir.AxisListType.X, op=mybir.AluOpType.add)
                # g = sqrt(t2) / (sigma^2 + t2); then v = s_u * g
                rt = sml.tile([128, 2, C], F32, tag="rt")
                nc.scalar.activation(out=rt[:], in_=t2[:],
                                     func=mybir.ActivationFunctionType.Sqrt)
                dn = sml.tile([128, 2, C], F32, tag="dn")
                nc.vector.tensor_tensor(
                    out=dn[:], in0=t2[:],
                    in1=sig2[:, None, :].to_broadcast((128, 2, C)),
                    op=mybir.AluOpType.add)
                nc.vector.reciprocal(out=dn[:], in_=dn[:])
                g_f = sml.tile([128, 2, C], F32, tag="g_f")
                nc.vector.tensor_tensor(out=g_f[:], in0=rt[:], in1=dn[:],
                                        op=mybir.AluOpType.mult)
                v_sb = sml.tile([128, 2, CO], MMDT, tag="v_sb")
                nc.vector.tensor_tensor(
                    out=v_sb[:].rearrange("p h (c o) -> p h c o", o=O),
                    in0=s_sb[:].rearrange("p h (c o) -> p h c o", o=O),
                    in1=g_f[:, :, :, None].to_broadcast((128, 2, C, O)),
                    op=mybir.AluOpType.mult)

                # ---- G = x^T v, then a = (1/B) sum_{i,o} W*G ----
                a_stage = sml.tile([16, NT, C], F32, tag="a_stage")
                for ch in range(6):
                    tsl = slice(3 * ch, 3 * ch + 3)
                    for t in range(3 * ch, 3 * ch + 3):
                        G_ps = ps_g.tile([128, CO], F32, tag="G_ps")
                        for h in range(2):
                            nc.tensor.matmul(
                                G_ps[:],
                                lhsT=xb_sb[:, h, 128 * t:128 * t + 128],
                                rhs=v_sb[:, h, :],
                                start=(h == 0), stop=(h == 1))
                        if t % 2 == 0:
                            nc.scalar.copy(out=Gbig[:, t, :], in_=G_ps[:])
                        else:
                            nc.vector.tensor_copy(out=Gbig[:, t, :], in_=G_ps[:])
                    nc.vector.tensor_tensor(out=Pp[:, tsl, :], in0=Gbig[:, tsl, :],
                                            in1=W_sb[:, tsl, :],
                                            op=mybir.AluOpType.mult)
                    nc.vector.tensor_reduce(
                        out=P2[:, tsl, :],
                        in_=Pp[:, tsl, :].rearrange("p t (c o) -> p t c o", o=O),
                        axis=mybir.AxisListType.X, op=mybir.AluOpType.add)
                    ap_ps = ps_m.tile([128, NT * C], F32, tag="ps_small", name="ap_ps")[0:16, 0:30]
                    nc.tensor.matmul(
                        ap_ps[:],
                        lhsT=oblk_sb[:],
                        rhs=P2[:, tsl, :].rearrange("p t c -> p (t c)"),
                        start=True, stop=True)
                    nc.scalar.copy(
                        out=a_stage[:, tsl, :],
                        in_=ap_ps[:].rearrange("m (t c) -> m t c", c=10))
                warm2 = sml.tile([1, 1], F32, tag="warm", name=f"warm_ex{it}")
                nc.scalar.activation(out=warm2[:], in_=a_stage[0:1, 0, 0:1],
                                     func=mybir.ActivationFunctionType.Exp)
                nc.vector.tensor_tensor(out=b_sb[:], in0=b_sb[:], in1=a_stage[:],
                                        op=mybir.AluOpType.add)

    nc.compile()
    return nc


def _get_nc():
    if _CACHE["nc"] is None:
        _CACHE["nc"] = _build()
    return _CACHE["nc"]


def _row_major(a4):
    """[T, j(16), i(8), ...] -> [T, i, j, ...] flattened rows: row = i*16+j."""
    return np.ascontiguousarray(a4.transpose(0, 2, 1, *range(3, a4.ndim)))


def kernel(x, W):
    global LAST_RESULT
    x = np.ascontiguousarray(np.asarray(x), dtype=np.float32)
    W = np.ascontiguousarray(np.asarray(W), dtype=np.float32)
    assert x.shape == (B, R, I) and W.shape == (R, C, O, I)

    nc = _get_nc()

    # [(r i), (c o)] with i-major row order within each 128-row tile
    Wp = W.transpose(0, 3, 1, 2).reshape(R // 16, 16, I, CO)
    Wp = _row_major(Wp).reshape(R * I, CO)
    oblk = np.zeros((128, 16), np.float32)
    for k in range(128):
        oblk[k, k % 16] = 1.0 / B      # row k = i*16+j -> j = k % 16
    o16 = np.ones((16, 1), np.float32)
    rep16 = np.zeros((16, 128), np.float32)
    for p in range(128):
        rep16[p % 16, p] = 1.0
    o1 = np.ones((1, 128), np.float32)

    import ml_dtypes
    mdt = ml_dtypes.bfloat16 if USE_BF16 else np.float32
    in_maps = []
    for cid in range(NCORE):
        xs = x[:, cid * RL:(cid + 1) * RL, :]                     # [B, 288, 8]
        xT = xs.transpose(1, 2, 0).reshape(NT, 16, I, B)          # [T, j, i, B]
        xT = _row_major(xT).reshape(KI, B)
        xbv = xs.reshape(B, NT, 16, I).transpose(0, 1, 3, 2)      # [B, T, i, j]
        xbv = np.ascontiguousarray(xbv).reshape(B, KI)
        in_maps.append({
            "xT": np.ascontiguousarray(xT, dtype=mdt),
            "xb": np.ascontiguousarray(xbv, dtype=mdt),
            "Wl": np.ascontiguousarray(Wp[cid * KI:(cid + 1) * KI], dtype=mdt),
            "ones_blk": oblk,
            "ones16": o16,
            "rep16": rep16,
            "ones1": np.ascontiguousarray(o1, dtype=mdt),
        })

    res = bass_utils.run_bass_kernel_spmd(
        nc, in_maps, core_ids=list(range(NCORE)),
        trace=bool(os.environ.get("DIGITCAPS_TRACE")))
    LAST_RESULT = res

    s2 = np.zeros((B, CO), np.float64)
    sig = np.zeros((10,), np.float64)
    for cid in range(NCORE):
        s2 += res.results[cid]["sout"]
        sig += res.results[cid]["sigout"][0, :10]
    s3 = (s2.reshape(B, C, O) / sig[None, :, None]).astype(np.float32)
    sq = (s3 * s3).sum(axis=2, keepdims=True)
    v = s3 * (np.sqrt(sq) / (1.0 + sq))
    return v[..., None].astype(np.float32)
